# revision 15
# baseline (speedup 1.0000x reference)
"""Trainium2 Bass kernel for nn_AdvancedFeatureExtractor (8-core SPMD).

Decomposition (validated against the jax reference in numpy):
  - time conv branch: one shared 128-tap im2col matmul per 512-col chunk
    (all three kernel sizes packed into one (128,84) weight matrix);
    BN batch stats via bn_stats/bn_aggr + cross-core AllGather.
  - freq branch: direct DFT as matmul, frequency-sliced across cores
    (core m computes bins [256m, 256m+256) for ALL 64 samples = band m);
    mag/phase band means exchanged via the same AllGather.
  - wavelet branch: 4-level db4 DWT via block-diagonal strided-conv matmul.
  - fusion MLP: const channels (stats/freq/wavelet feats, constant over L)
    folded into a per-sample bias; only the 84 conv channels go through the
    big per-position matmuls.
"""
import sys
import os
import math

for _p in ("/opt/trn_rl_repo",):
    if _p not in sys.path:
        sys.path.insert(0, _p)

import numpy as np
import ml_dtypes

import concourse.bass as bass
import concourse.mybir as mybir
import concourse.tile as tile
from concourse.bass_types import AP
from concourse.vector_clock import ScopedClock

F32 = mybir.dt.float32
BF16 = mybir.dt.bfloat16
AF = mybir.ActivationFunctionType
OP = mybir.AluOpType
BF = ml_dtypes.bfloat16

L = 4094
B = 64
NCORES = 8
BS = B // NCORES            # 8 samples per core
CH = 512                    # free-dim chunk
NCH_S = 8                   # chunks per sample (last is 510 wide)
LASTW = L - (NCH_S - 1) * CH  # 510
NLOC = BS * L               # per-core BN element count = 32752
NGLOB = B * L               # 262016
BANDW = 256                 # bins per band (= per core)
PI = math.pi

# DWT level geometry
DWT_NIN = [4094, 2051, 1029, 518]
DWT_NOUT = [2051, 1029, 518, 263]
DWT_NPAD = [n + 14 for n in DWT_NIN]
DWT_WE = [(n + 1) // 2 for n in DWT_NPAD]   # even-index width
DWT_WO = [n // 2 for n in DWT_NPAD]         # odd-index width

_DB4_LO = np.array([-0.010597401784997278, 0.032883011666982945, 0.030841381835986965,
                    -0.18703481171888114, -0.02798376941698385, 0.6308807679295904,
                    0.7148465705525415, 0.23037781330885523], np.float32)
_DB4_HI = np.array([-0.23037781330885523, 0.7148465705525415, -0.6308807679295904,
                    -0.02798376941698385, 0.18703481171888114, 0.030841381835986965,
                    0.032883011666982945, -0.010597401784997278], np.float32)
WIN = (32, 64, 128)

_MAXW = 1


def _patched_drain_and_barrier(self, tick_clock, wait_clock):
    """walrus in this container rejects >1 sem wait on a CTRL instruction;
    split the Tile tail-drain waits across single-wait NOPs."""
    nc = self.nc
    probe = nc.sync.nop()
    wait_clock.add_sem_waits(probe.ins, ScopedClock({None: tick_clock.global_clock}))
    si = probe.ins.sync_info
    waits = list(si.on_wait) if si is not None else []
    updates = list(si.on_update) if si is not None else []
    probe.ins.sync_info = mybir.SyncInfo(on_wait=waits[:_MAXW], on_update=updates)
    for i in range(_MAXW, len(waits), _MAXW):
        nop = nc.sync.nop()
        nop.ins.sync_info = mybir.SyncInfo(on_wait=waits[i:i + _MAXW], on_update=[])
    nc.sync.drain()
    nc.all_engine_barrier()
    assert self.sems is not None
    popped = nc._tile_sem_poison_stack.pop()
    assert popped is self._sem_poison
    _sems = list(self.sems.allocated().values())
    _nums = sorted(s.num if hasattr(s, "num") else s for s in _sems)
    for _i in range(0, len(_nums), 8):
        chunk = _nums[_i:_i + 8]
        # only contiguous runs within the chunk
        run = [chunk[0]]
        for n in chunk[1:]:
            if n == run[-1] + 1:
                run.append(n)
            else:
                nc.gpsimd.dma_reset(range(run[0], run[-1] + 1))
                nc.gpsimd.sem_clear(range(run[0], run[-1] + 1))
                run = [n]
        nc.gpsimd.dma_reset(range(run[0], run[-1] + 1))
        nc.gpsimd.sem_clear(range(run[0], run[-1] + 1))
    nc._state.prepend_free_semaphores(_nums)
    for poison_set in nc._tile_sem_poison_stack:
        poison_set.update(_nums)
    nc.all_engine_barrier()


tile.TileContext._drain_and_barrier = _patched_drain_and_barrier


def _split_multi_waits(nc):
    """This walrus build allows only one sem-wait per instruction: hoist extra
    waits onto same-engine NOPs inserted immediately before the instruction."""
    for f in nc.m.functions:
        for bb in f.blocks:
            insts = bb.instructions
            i = 0
            while i < len(insts):
                ins = insts[i]
                si = getattr(ins, "sync_info", None)
                if si is not None and len(si.on_wait) > 1:
                    waits = list(si.on_wait)
                    for j, wsub in enumerate(waits[:-1]):
                        nop = mybir.InstNoOp(
                            name=f"{ins.name}.w{j}", engine=ins.engine,
                            bass_nofuse=True,
                            sync_info=mybir.SyncInfo(on_wait=[wsub], on_update=[]))
                        insts.insert(i, nop)
                        i += 1
                    ins.sync_info = mybir.SyncInfo(
                        on_wait=[waits[-1]], on_update=list(si.on_update))
                i += 1


def _ap(t, offset, dims):
    """Custom access pattern on a tile/param AP `t` (adds t's own offset)."""
    return AP(tensor=t.tensor, offset=t.offset + offset, ap=[list(d) for d in dims])


def build(n_iters: int = 1, phase: float = 5) -> bass.Bass:
    nc = bass.Bass()

    # ---- per-core external inputs ----
    xpad = nc.declare_dram_parameter("xpad", [BS, 4224], BF16, isOutput=False)
    xdwt = nc.declare_dram_parameter("xdwt", [BS, 4108], F32, isOutput=False)
    xt = nc.declare_dram_parameter("xt", [32, 128, 64], BF16, isOutput=False)
    dmat = nc.declare_dram_parameter("dmat", [32, 128, 512], BF16, isOutput=False)
    wall = nc.declare_dram_parameter("wall", [128, 84], BF16, isOutput=False)
    wdwt = nc.declare_dram_parameter("wdwt", [64, 16], F32, isOutput=False)
    w1v = nc.declare_dram_parameter("w1v", [84, 256], BF16, isOutput=False)
    w2p = nc.declare_dram_parameter("w2p", [128, 512], BF16, isOutput=False)
    w1cS = nc.declare_dram_parameter("w1cS", [28, 256], F32, isOutput=False)
    w1cM = nc.declare_dram_parameter("w1cM", [85, 256], F32, isOutput=False)
    w1cP = nc.declare_dram_parameter("w1cP", [85, 256], F32, isOutput=False)
    w1cW = nc.declare_dram_parameter("w1cW", [85, 256], F32, isOutput=False)
    fub1 = nc.declare_dram_parameter("fub1", [128, 2], F32, isOutput=False)
    fub2 = nc.declare_dram_parameter("fub2", [128, 2], F32, isOutput=False)
    stw = nc.declare_dram_parameter("stw", [4, 28], F32, isOutput=False)
    stb = nc.declare_dram_parameter("stb", [28], F32, isOutput=False)
    mlpw = {}
    for pre in ("mag", "ph", "wv"):
        mlpw[pre] = (
            nc.declare_dram_parameter(f"{pre}w1", [8, 85], F32, isOutput=False),
            nc.declare_dram_parameter(f"{pre}b1", [85], F32, isOutput=False),
            nc.declare_dram_parameter(f"{pre}w2", [85, 85], F32, isOutput=False),
            nc.declare_dram_parameter(f"{pre}b2", [85], F32, isOutput=False),
        )
    sel = nc.declare_dram_parameter("sel", [64, 8], F32, isOutput=False)
    eye = nc.declare_dram_parameter("eye", [128, 128], F32, isOutput=False)
    bng = nc.declare_dram_parameter("bng", [84], F32, isOutput=False)
    bnb = nc.declare_dram_parameter("bnb", [84], F32, isOutput=False)
    out_ext = nc.declare_dram_parameter("out", [BS, 256, L], F32, isOutput=True)

    with tile.TileContext(nc) as tc:
        with tc.tile_pool(name="consts", bufs=1) as cpool, \
             tc.tile_pool(name="ystore", bufs=1) as ypool, \
             tc.tile_pool(name="xck", bufs=4) as xpool, \
             tc.tile_pool(name="dck", bufs=3) as dpool, \
             tc.tile_pool(name="ynp", bufs=3) as ynpool, \
             tc.tile_pool(name="h1p", bufs=4) as h1pool, \
             tc.tile_pool(name="ocp", bufs=4) as ocpool, \
             tc.tile_pool(name="small", bufs=1) as spool, \
             tc.tile_pool(name="dwtsb", bufs=1) as wpool, \
             tc.tile_pool(name="ps", bufs=5, space="PSUM") as ps, \
             tc.tile_pool(name="psdft", bufs=1, space="PSUM") as psdft, \
             tc.tile_pool(name="psmlp", bufs=2, space="PSUM") as psmlp, \
             tc.tile_pool(name="dram", bufs=2, space="DRAM") as dram:

            # ================= constants =================
            wall_sb = cpool.tile([128, 84], BF16)
            nc.gpsimd.dma_start(wall_sb[:], wall[:])
            xt_sb = cpool.tile([128, 2048], BF16)
            for c in range(32):
                nc.gpsimd.dma_start(xt_sb[:, 64 * c:64 * c + 64], xt[c])
            w1v_sb = cpool.tile([84, 256], BF16)
            nc.gpsimd.dma_start(w1v_sb[:], w1v[:])
            w2_sb = cpool.tile([128, 512], BF16)
            nc.gpsimd.dma_start(w2_sb[:], w2p[:])
            w1cS_sb = cpool.tile([28, 256], F32)
            nc.gpsimd.dma_start(w1cS_sb[:], w1cS[:])
            w1cM_sb = cpool.tile([85, 256], F32)
            nc.gpsimd.dma_start(w1cM_sb[:], w1cM[:])
            w1cP_sb = cpool.tile([85, 256], F32)
            nc.gpsimd.dma_start(w1cP_sb[:], w1cP[:])
            w1cW_sb = cpool.tile([85, 256], F32)
            nc.gpsimd.dma_start(w1cW_sb[:], w1cW[:])
            fub1_sb = cpool.tile([128, 2], F32)
            nc.gpsimd.dma_start(fub1_sb[:], fub1[:])
            fub2_sb = cpool.tile([128, 2], F32)
            nc.gpsimd.dma_start(fub2_sb[:], fub2[:])
            stw_sb = cpool.tile([4, 28], F32)
            nc.gpsimd.dma_start(stw_sb[:], stw[:])
            stb_sb = cpool.tile([28, 1], F32)
            nc.gpsimd.dma_start(stb_sb[:], _ap(stb[:], 0, [[1, 28], [1, 1]]))
            mlp_sb = {}
            for pre in ("mag", "ph", "wv"):
                p1, pb1, p2, pb2 = mlpw[pre]
                w1_sb = cpool.tile([8, 85], F32, name=f"{pre}w1sb")
                nc.gpsimd.dma_start(w1_sb[:], p1[:])
                b1_sb = cpool.tile([85, 1], F32, name=f"{pre}b1sb")
                nc.gpsimd.dma_start(b1_sb[:], _ap(pb1[:], 0, [[1, 85], [1, 1]]))
                w2_sb2 = cpool.tile([85, 85], F32, name=f"{pre}w2sb")
                nc.gpsimd.dma_start(w2_sb2[:], p2[:])
                b2_sb = cpool.tile([85, 1], F32, name=f"{pre}b2sb")
                nc.gpsimd.dma_start(b2_sb[:], _ap(pb2[:], 0, [[1, 85], [1, 1]]))
                mlp_sb[pre] = (w1_sb, b1_sb, w2_sb2, b2_sb)
            bng_sb = cpool.tile([84, 1], F32)
            nc.gpsimd.dma_start(bng_sb[:], _ap(bng[:], 0, [[1, 84], [1, 1]]))
            bnb_sb = cpool.tile([84, 1], F32)
            nc.gpsimd.dma_start(bnb_sb[:], _ap(bnb[:], 0, [[1, 84], [1, 1]]))
            wdwt_sb = cpool.tile([64, 16], F32)
            nc.gpsimd.dma_start(wdwt_sb[:], wdwt[:])
            ones8 = cpool.tile([8, 1], F32)
            nc.vector.memset(ones8[:], 1.0)
            sel_sb = cpool.tile([64, 8], F32)
            nc.gpsimd.dma_start(sel_sb[:], sel[:])
            eye_sb = cpool.tile([128, 128], F32)
            nc.gpsimd.dma_start(eye_sb[:], eye[:])

            for it in range(n_iters):
                # ============ conv branch: im2col matmul + bn stats ============
                y_store = ypool.tile([84, NCH_S * BS * CH], BF16)
                bnstat = spool.tile([84, 6 * NCH_S * BS], F32)
                for s in range(BS):
                    for ci in range(NCH_S):
                        idx = s * NCH_S + ci
                        w = LASTW if ci == NCH_S - 1 else CH
                        xc = xpool.tile([128, CH], BF16)
                        nc.sync.dma_start(
                            xc[:], _ap(xpad[:], s * 4224 + ci * CH, [[1, 128], [1, CH]]))
                        yp = ps.tile([84, CH], F32, tag="bigps")
                        nc.tensor.matmul(yp[:], wall_sb[:], xc[:])
                        nc.scalar.copy(y_store[:, idx * CH:(idx + 1) * CH], yp[:])
                        nc.vector.bn_stats(bnstat[:, idx * 6:idx * 6 + 6], yp[:, :w])
                bnmv = spool.tile([84, 2], F32)
                nc.vector.bn_aggr(bnmv[:], bnstat[:])

                if phase < 1.2:
                    continue
                # ============ DFT branch (own 256 bins, all 64 samples) ============
                dacc = psdft.tile([64, 512], F32)
                for c in range(32):
                    dt_ = dpool.tile([128, 512], BF16)
                    nc.sync.dma_start(dt_[:], dmat[c])
                    nc.tensor.matmul(dacc[:], xt_sb[:, c * 64:(c + 1) * 64], dt_[:],
                                     start=(c == 0), stop=(c == 31))
                if phase < 1.3:
                    continue
                bufs8 = [spool.tile([64, 256], F32, name=f"dp{i}") for i in range(8)]
                b1, b2, b3, b4, b5, b6, b7, b8 = [t[:] for t in bufs8]
                nc.vector.tensor_copy(b1, dacc[:, 0:256])   # re -> SBUF
                nc.vector.tensor_copy(b2, dacc[:, 256:512])  # im -> SBUF
                nc.vector.tensor_tensor(b3, b1, b1, OP.mult)
                nc.vector.tensor_tensor(b4, b2, b2, OP.mult)
                nc.vector.tensor_tensor(b5, b3, b4, OP.add)
                nc.scalar.sqrt(b6, b5)
                magm = spool.tile([64, 1], F32)
                nc.vector.tensor_reduce(magm[:], b6, mybir.AxisListType.X, OP.add)
                magmean = spool.tile([64, 1], F32)
                nc.scalar.mul(magmean[:], magm[:], 1.0 / BANDW)
                if phase < 1.6:
                    continue
                # atan2(im, re); arctan LUT domain is [-pi/2, pi/2] so range-reduce
                nc.scalar.activation(b3, b2, AF.Abs)  # |im|
                nc.scalar.activation(b4, b1, AF.Abs)  # |re|
                nc.vector.tensor_tensor(b5, b3, b4, OP.min)
                nc.vector.tensor_tensor(b6, b3, b4, OP.max)
                nc.vector.reciprocal(b7, b6)
                nc.vector.tensor_tensor(b8, b5, b7, OP.mult)
                nc.scalar.activation(b5, b8, AF.Arctan)
                nc.vector.tensor_tensor(b6, b3, b4, OP.is_gt)   # swap flag
                nc.vector.tensor_scalar(b7, b6, -2.0, 1.0, OP.mult, OP.add)
                nc.vector.tensor_tensor(b8, b5, b7, OP.mult)
                nc.vector.scalar_tensor_tensor(b7, b6, PI / 2, b8, OP.mult, OP.add)  # ang0
                nc.vector.tensor_scalar(b3, b1, 0.0, None, OP.is_lt)   # re<0
                nc.vector.tensor_scalar(b4, b3, -2.0, 1.0, OP.mult, OP.add)
                nc.vector.tensor_tensor(b5, b7, b4, OP.mult)
                nc.vector.scalar_tensor_tensor(b6, b3, PI, b5, OP.mult, OP.add)      # ang1
                nc.vector.tensor_scalar(b4, b2, 0.0, None, OP.is_ge)
                nc.vector.tensor_scalar(b5, b4, 2.0, -1.0, OP.mult, OP.add)          # sign(im)
                nc.vector.tensor_tensor(b8, b6, b5, OP.mult)
                phm = spool.tile([64, 1], F32)
                nc.vector.tensor_reduce(phm[:], b8, mybir.AxisListType.X, OP.add)
                phmean = spool.tile([64, 1], F32)
                nc.scalar.mul(phmean[:], phm[:], 1.0 / BANDW)

                if phase < 1.8:
                    continue
                # ============ BN partial sums -> AllGather row ============
                bsum = spool.tile([84, 1], F32)
                nc.scalar.mul(bsum[:], bnmv[:, 0:1], float(NLOC))
                msq_ = spool.tile([84, 1], F32)
                nc.vector.tensor_tensor(msq_[:], bnmv[:, 0:1], bnmv[:, 0:1], OP.mult)
                vps = spool.tile([84, 1], F32)
                nc.vector.tensor_tensor(vps[:], bnmv[:, 1:2], msq_[:], OP.add)
                bsq = spool.tile([84, 1], F32)
                nc.vector.tensor_scalar(bsq[:], vps[:], float(NLOC), None, OP.mult)

                ag_src = dram.tile([1, 296], F32)
                ag_dst = dram.tile([8, 296], F32, addr_space="Shared")
                nc.gpsimd.dma_start(_ap(ag_src[:], 0, [[1, 64], [1, 1]]), magmean[:])
                nc.gpsimd.dma_start(_ap(ag_src[:], 64, [[1, 64], [1, 1]]), phmean[:])
                nc.gpsimd.dma_start(_ap(ag_src[:], 128, [[1, 84], [1, 1]]), bsum[:])
                nc.gpsimd.dma_start(_ap(ag_src[:], 212, [[1, 84], [1, 1]]), bsq[:])

                if phase < 3:
                    continue
                # ============ x-stats + DWT (overlap with collective) ============
                apad0 = wpool.tile([BS, 4108], F32, name="apad0")
                nc.gpsimd.dma_start(apad0[:], xdwt[:])
                xrow = apad0[:, 7:7 + L]
                xjunk = wpool.tile([BS, L], BF16, name="xjunk")
                xsum = spool.tile([BS, 1], F32)
                nc.scalar.activation(xjunk[:], xrow, AF.Copy, accum_out=xsum[:])
                xsqs = spool.tile([BS, 1], F32)
                nc.scalar.activation(xjunk[:], xrow, AF.Square, accum_out=xsqs[:])
                xmax = spool.tile([BS, 1], F32)
                nc.vector.tensor_reduce(xmax[:], xrow, mybir.AxisListType.X, OP.max)
                xmin = spool.tile([BS, 1], F32)
                nc.vector.tensor_reduce(xmin[:], xrow, mybir.AxisListType.X, OP.min)
                xmean = spool.tile([BS, 1], F32)
                nc.scalar.mul(xmean[:], xsum[:], 1.0 / L)
                xmsq = spool.tile([BS, 1], F32)
                nc.vector.tensor_tensor(xmsq[:], xmean[:], xmean[:], OP.mult)
                xu = spool.tile([BS, 1], F32)
                nc.vector.tensor_scalar(xu[:], xmsq[:], -float(L) / (L - 1), None, OP.mult)
                xv_ = spool.tile([BS, 1], F32)
                nc.vector.scalar_tensor_tensor(xv_[:], xsqs[:], 1.0 / (L - 1), xu[:], OP.mult, OP.add)
                xstd = spool.tile([BS, 1], F32)
                nc.scalar.sqrt(xstd[:], xv_[:])
                statdram = dram.tile([4, 8], F32)
                for r, tl in enumerate((xmean, xstd, xmax, xmin)):
                    nc.gpsimd.dma_start(_ap(statdram[:], 8 * r, [[1, 8], [1, 1]]), tl[:])

                # DWT levels
                wf_dram = dram.tile([8, 16], F32)
                apad_cur = apad0  # (8, 4108) level-0 padded input
                for lv in range(4):
                    nin, nout = DWT_NIN[lv], DWT_NOUT[lv]
                    npad, we, wo = DWT_NPAD[lv], DWT_WE[lv], DWT_WO[lv]
                    nrow = BS if lv == 0 else 16
                    # deinterleave into even/odd, bounce through DRAM
                    ae = wpool.tile([nrow, we], F32, name=f"ae{lv}", tag="aet")
                    nc.vector.tensor_copy(
                        ae[:], _ap(apad_cur[:], 0, [[apad_cur.ap[0][0], nrow], [2, we]]))
                    ao = wpool.tile([nrow, wo], F32, name=f"ao{lv}", tag="aot")
                    nc.vector.tensor_copy(
                        ao[:], _ap(apad_cur[:], 1, [[apad_cur.ap[0][0], nrow], [2, wo]]))
                    ae_d = dram.tile([nrow, we], F32, name=f"aed{lv}")
                    nc.gpsimd.dma_start(ae_d[:], ae[:])
                    ao_d = dram.tile([nrow, wo], F32, name=f"aod{lv}")
                    nc.gpsimd.dma_start(ao_d[:], ao[:])
                    xlv = wpool.tile([64, nout], F32, name=f"xlv{lv}", tag="xlv")
                    rstep = we if lv == 0 else 2 * we
                    rstepo = wo if lv == 0 else 2 * wo
                    for t in range(8):
                        srcd = ae_d if t % 2 == 0 else ao_d
                        rst = rstep if t % 2 == 0 else rstepo
                        nc.sync.dma_start(
                            xlv[8 * t:8 * t + 8, :],
                            _ap(srcd[:], t // 2, [[rst, 8], [1, nout]]))
                    # next-level padded buffer (evac target)
                    nch = (nout + CH - 1) // CH
                    if lv < 3:
                        apad_next = wpool.tile([16, DWT_NPAD[lv + 1]], F32, name=f"apad{lv + 1}", tag="apadA" if (lv + 1) % 2 == 0 else "apadB")
                        evtgt = apad_next
                        evoff = 7
                    else:
                        evtgt = wpool.tile([16, nout], F32, name="apadj", tag="apadA")
                        evoff = 0
                    csums = spool.tile([16, 8], F32, name=f"cs{lv}")
                    cabss = spool.tile([16, 8], F32, name=f"ca{lv}")
                    csqs = spool.tile([16, 8], F32, name=f"cq{lv}")
                    for c in range(nch):
                        c0 = c * CH
                        wch = min(CH, nout - c0)
                        dp = ps.tile([16, CH], F32, tag="bigps", name=f"dwtp{lv}")
                        nc.tensor.matmul(dp[:, :wch], wdwt_sb[:], xlv[:, c0:c0 + wch])
                        nc.scalar.activation(evtgt[:, evoff + c0:evoff + c0 + wch],
                                             dp[:, :wch], AF.Copy,
                                             accum_out=csums[:, c:c + 1])
                        junka = wpool.tile([16, CH], BF16, name="junka")
                        nc.scalar.activation(junka[:, :wch], dp[:, :wch], AF.Abs,
                                             accum_out=cabss[:, c:c + 1])
                        junkb = wpool.tile([16, CH], F32, name="junkb")
                        cad = evtgt[:, evoff + c0:evoff + c0 + wch]
                        nc.vector.tensor_tensor(junkb[:, :wch], cad, cad, OP.mult)
                        nc.vector.tensor_reduce(csqs[:, c:c + 1], junkb[:, :wch],
                                                mybir.AxisListType.X, OP.add)
                    if lv < 3:
                        # symmetric pads for next level
                        npn = DWT_NPAD[lv + 1]
                        nc.vector.tensor_copy(
                            apad_next[:, 0:7],
                            _ap(apad_next[:], 13, [[apad_next.ap[0][0], 16], [-1, 7]]))
                        nc.vector.tensor_copy(
                            apad_next[:, 7 + nout:npn],
                            _ap(apad_next[:], 7 + nout - 1,
                                [[apad_next.ap[0][0], 16], [-1, 7]]))
                        apad_cur = apad_next
                    # reduce chunk stats -> level features
                    ctot = spool.tile([16, 1], F32, name=f"ct{lv}")
                    nc.vector.tensor_reduce(ctot[:], csums[:, :nch], mybir.AxisListType.X, OP.add)
                    atot = spool.tile([16, 1], F32, name=f"at{lv}")
                    nc.vector.tensor_reduce(atot[:], cabss[:, :nch], mybir.AxisListType.X, OP.add)
                    qtot = spool.tile([16, 1], F32, name=f"qt{lv}")
                    nc.vector.tensor_reduce(qtot[:], csqs[:, :nch], mybir.AxisListType.X, OP.add)
                    man = spool.tile([16, 1], F32, name=f"man{lv}")
                    nc.scalar.mul(man[:], atot[:], 1.0 / nout)
                    cmean = spool.tile([16, 1], F32, name=f"cm{lv}")
                    nc.scalar.mul(cmean[:], ctot[:], 1.0 / nout)
                    cmsq = spool.tile([16, 1], F32, name=f"cmq{lv}")
                    nc.vector.tensor_tensor(cmsq[:], cmean[:], cmean[:], OP.mult)
                    cvar = spool.tile([16, 1], F32, name=f"cv{lv}")
                    nc.vector.scalar_tensor_tensor(cvar[:], qtot[:], 1.0 / nout, cmsq[:],
                                                   OP.mult, OP.subtract)
                    cstd = spool.tile([16, 1], F32, name=f"cd{lv}")
                    nc.scalar.sqrt(cstd[:], cvar[:])
                    r0 = 2 * (3 - lv)
                    nc.gpsimd.dma_start(_ap(wf_dram[:], 16 * r0, [[1, 16], [1, 1]]), man[:])
                    nc.gpsimd.dma_start(_ap(wf_dram[:], 16 * (r0 + 1), [[1, 16], [1, 1]]), cstd[:])

                if phase < 4:
                    continue
                # ============ collective ============
                nc.gpsimd.collective_compute(
                    "AllGather", OP.bypass,
                    replica_groups=[list(range(NCORES))],
                    ins=[ag_src.opt()], outs=[ag_dst.opt()])

                # ============ post-collective: BN + const features ============
                gath = spool.tile([8, 296], F32)
                nc.gpsimd.dma_start(gath[:], ag_dst[:])
                gsp = psmlp.tile([1, 296], F32, tag="mlpp")
                nc.tensor.matmul(gsp[:], ones8[:], gath[:])
                gsums = spool.tile([1, 296], F32)
                nc.vector.tensor_copy(gsums[:], gsp[:])
                gb_d = dram.tile([1, 296], F32)
                nc.gpsimd.dma_start(gb_d[:], gsums[:])
                gs84 = spool.tile([84, 1], F32)
                nc.gpsimd.dma_start(gs84[:], _ap(gb_d[:], 128, [[1, 84], [1, 1]]))
                gq84 = spool.tile([84, 1], F32)
                nc.gpsimd.dma_start(gq84[:], _ap(gb_d[:], 212, [[1, 84], [1, 1]]))
                bmean = spool.tile([84, 1], F32)
                nc.scalar.mul(bmean[:], gs84[:], 1.0 / NGLOB)
                bmneg = spool.tile([84, 1], F32)
                nc.scalar.mul(bmneg[:], gs84[:], -1.0 / NGLOB)
                bmsq = spool.tile([84, 1], F32)
                nc.vector.tensor_tensor(bmsq[:], bmean[:], bmneg[:], OP.mult)
                bvar = spool.tile([84, 1], F32)
                nc.vector.scalar_tensor_tensor(bvar[:], gq84[:], 1.0 / NGLOB, bmsq[:],
                                               OP.mult, OP.add)
                bve = spool.tile([84, 1], F32)
                nc.vector.tensor_scalar(bve[:], bvar[:], 1e-5, None, OP.add)
                bsd = spool.tile([84, 1], F32)
                nc.scalar.sqrt(bsd[:], bve[:])
                brq = spool.tile([84, 1], F32)
                nc.vector.reciprocal(brq[:], bsd[:])
                bnscale = spool.tile([84, 1], F32)
                nc.vector.tensor_tensor(bnscale[:], brq[:], bng_sb[:], OP.mult)
                bnms = spool.tile([84, 1], F32)
                nc.vector.tensor_tensor(bnms[:], bmneg[:], bnscale[:], OP.mult)
                bnbias = spool.tile([84, 1], F32)
                nc.vector.tensor_tensor(bnbias[:], bnms[:], bnb_sb[:], OP.add)

                statsT = spool.tile([4, 8], F32)
                nc.gpsimd.dma_start(statsT[:], statdram[:])
                wfT = spool.tile([8, 8], F32)
                for s_ in range(8):
                    nc.gpsimd.dma_start(
                        wfT[:, s_:s_ + 1],
                        _ap(wf_dram[:], 2 * s_ + 1, [[16, 8], [1, 1]]))

                cfS = spool.tile([28, 8], F32)
                sfp = psmlp.tile([28, 8], F32, tag="mlpp")
                nc.tensor.matmul(sfp[:], stw_sb[:], statsT[:])
                nc.scalar.activation(cfS[:], sfp[:], AF.Identity, bias=stb_sb[:])
                # band MLPs: all-64-sample layer-1, select own samples via
                # one-hot matmul (selection commutes with bias/relu), then
                # PE-transpose so biases become per-partition.
                cfM = spool.tile([85, 8], F32)
                cfP = spool.tile([85, 8], F32)
                cfW = spool.tile([85, 8], F32)
                for pre, goff, tgt in (("mag", 0, cfM), ("ph", 64, cfP)):
                    w1_sb, b1_sb, w2_sb2, b2_sb = mlp_sb[pre]
                    h64p = psmlp.tile([64, 85], F32, tag="mlpp", name=f"h64p{pre}")
                    nc.tensor.matmul(h64p[:], gath[0:8, goff:goff + 64], w1_sb[:])
                    h64 = spool.tile([64, 85], F32, name=f"h64{pre}")
                    nc.vector.tensor_copy(h64[:], h64p[:])
                    hselp = psmlp.tile([8, 85], F32, tag="mlpp", name=f"hsel{pre}")
                    nc.tensor.matmul(hselp[:], sel_sb[:], h64[:])
                    hsel = spool.tile([8, 85], F32, name=f"hselS{pre}")
                    nc.vector.tensor_copy(hsel[:], hselp[:])
                    htp = psmlp.tile([85, 8], F32, tag="mlpp", name=f"htp{pre}")
                    nc.tensor.transpose(htp[:], hsel[:], eye_sb[0:8, 0:8])
                    ht = spool.tile([85, 8], F32, name=f"ht{pre}")
                    nc.scalar.activation(ht[:], htp[:], AF.Relu, bias=b1_sb[:])
                    op2 = psmlp.tile([85, 8], F32, tag="mlpp", name=f"op{pre}")
                    nc.tensor.matmul(op2[:], w2_sb2[:], ht[:])
                    nc.scalar.activation(tgt[:], op2[:], AF.Identity, bias=b2_sb[:])
                # wavelet MLP (local samples, simple orientation)
                w1_sb, b1_sb, w2_sb2, b2_sb = mlp_sb["wv"]
                hpw = psmlp.tile([85, 8], F32, tag="mlpp", name="hpwv")
                nc.tensor.matmul(hpw[:], w1_sb[:], wfT[:])
                hhw = spool.tile([85, 8], F32, name="hhwv")
                nc.scalar.activation(hhw[:], hpw[:], AF.Relu, bias=b1_sb[:])
                opw = psmlp.tile([85, 8], F32, tag="mlpp", name="opwv")
                nc.tensor.matmul(opw[:], w2_sb2[:], hhw[:])
                nc.scalar.activation(cfW[:], opw[:], AF.Identity, bias=b2_sb[:])
                # const-channel contribution to layer-1 bias
                cbT = []
                for oh in range(2):
                    cbp = psmlp.tile([128, 8], F32, tag="mlpp", name=f"cbp{oh}")
                    sl = slice(oh * 128, oh * 128 + 128)
                    nc.tensor.matmul(cbp[:], w1cS_sb[:, sl], cfS[:], start=True, stop=False)
                    nc.tensor.matmul(cbp[:], w1cM_sb[:, sl], cfM[:], start=False, stop=False)
                    nc.tensor.matmul(cbp[:], w1cP_sb[:, sl], cfP[:], start=False, stop=False)
                    nc.tensor.matmul(cbp[:], w1cW_sb[:, sl], cfW[:], start=False, stop=True)
                    cb = spool.tile([128, 8], F32, name=f"cbT{oh}")
                    nc.scalar.activation(cb[:], cbp[:], AF.Identity,
                                         bias=fub1_sb[:, oh:oh + 1])
                    cbT.append(cb)

                if phase < 5:
                    continue
                # ============ fusion ============
                for s in range(BS):
                    for ci in range(NCH_S):
                        idx = s * NCH_S + ci
                        w = LASTW if ci == NCH_S - 1 else CH
                        yn = ynpool.tile([84, CH], BF16)
                        nc.scalar.activation(yn[:], y_store[:, idx * CH:(idx + 1) * CH],
                                             AF.Relu, bias=bnbias[:], scale=bnscale[:])
                        h1s = []
                        for oh in range(2):
                            hp1 = ps.tile([128, CH], F32, tag="bigps", name=f"hps{oh}")
                            nc.tensor.matmul(hp1[:], w1v_sb[:, oh * 128:(oh + 1) * 128], yn[:])
                            h1 = h1pool.tile([128, CH], BF16, name=f"h1{oh}")
                            if oh == 0:
                                nc.scalar.activation(h1[:], hp1[:], AF.Relu,
                                                     bias=cbT[0][:, s:s + 1])
                            else:
                                nc.vector.tensor_scalar(h1[:], hp1[:], cbT[1][:, s:s + 1],
                                                        0.0, OP.add, OP.max)
                            h1s.append(h1)
                        for of in range(2):
                            op_ = ps.tile([128, CH], F32, tag="bigps", name=f"ops{of}")
                            nc.tensor.matmul(op_[:], w2_sb[:, (0 * 2 + of) * 128:(0 * 2 + of) * 128 + 128],
                                             h1s[0][:], start=True, stop=False)
                            nc.tensor.matmul(op_[:], w2_sb[:, (1 * 2 + of) * 128:(1 * 2 + of) * 128 + 128],
                                             h1s[1][:], start=False, stop=True)
                            oc = ocpool.tile([128, CH], F32, name=f"oc{of}")
                            if of == 0:
                                nc.scalar.activation(oc[:], op_[:], AF.Identity,
                                                     bias=fub2_sb[:, 0:1])
                            else:
                                nc.vector.tensor_scalar(oc[:], op_[:], fub2_sb[:, 1:2],
                                                        None, OP.add)
                            nc.sync.dma_start(
                                _ap(out_ext[:], s * 256 * L + (of * 128) * L + ci * CH,
                                    [[L, 128], [1, w]]),
                                oc[:, :w])
    _split_multi_waits(nc)
    return nc


def pack_inputs(inputs: dict) -> list[dict]:
    x = np.asarray(inputs["x"], np.float32)
    fu_w1 = np.asarray(inputs["fu_w1"], np.float32)
    fu_w2 = np.asarray(inputs["fu_w2"], np.float32)

    # shared (replicated) tensors
    wall = np.zeros((128, 84), np.float32)
    for i, k in enumerate(WIN):
        w = np.asarray(inputs[f"tc_w{i}"], np.float32)[:, 0, :]  # (28, k)
        p0 = 64 - k // 2
        wall[p0:p0 + k, i * 28:(i + 1) * 28] = w.T
    wall = wall.astype(BF)

    lo = _DB4_LO[::-1].copy()
    hi = _DB4_HI[::-1].copy()
    wdwt = np.zeros((64, 16), np.float32)
    for t in range(8):
        for s in range(8):
            wdwt[8 * t + s, 2 * s] = lo[t]
            wdwt[8 * t + s, 2 * s + 1] = hi[t]

    xtfull = np.zeros((4096, 64), np.float32)
    xtfull[:L] = x.T
    xt = xtfull.astype(BF).reshape(32, 128, 64)

    w1v = fu_w1[:84].astype(BF)                        # (84, 256)
    w2p = np.zeros((128, 512), np.float32)
    for kh in range(2):
        for oh in range(2):
            w2p[:, (kh * 2 + oh) * 128:(kh * 2 + oh) * 128 + 128] = \
                fu_w2[kh * 128:(kh + 1) * 128, oh * 128:(oh + 1) * 128]
    w2p = w2p.astype(BF)

    w1cS = fu_w1[84:112].copy()     # sf
    w1cM = fu_w1[112:197].copy()    # mag
    w1cP = fu_w1[197:282].copy()    # ph
    w1cW = fu_w1[282:367].copy()    # wf
    fub1 = np.stack([np.asarray(inputs["fu_b1"], np.float32)[:128],
                     np.asarray(inputs["fu_b1"], np.float32)[128:]], axis=1)
    fub2 = np.stack([np.asarray(inputs["fu_b2"], np.float32)[:128],
                     np.asarray(inputs["fu_b2"], np.float32)[128:]], axis=1)

    shared = {
        "eye": np.eye(128, dtype=np.float32),
        "xt": xt, "wall": wall, "wdwt": wdwt.astype(np.float32),
        "w1v": w1v, "w2p": w2p, "w1cS": w1cS, "w1cM": w1cM, "w1cP": w1cP, "w1cW": w1cW,
        "fub1": fub1, "fub2": fub2,
        "stw": np.asarray(inputs["st_w"], np.float32),
        "stb": np.asarray(inputs["st_b"], np.float32),
        "bng": np.concatenate([np.asarray(inputs[f"bn_g{i}"], np.float32) for i in range(3)]),
        "bnb": np.concatenate([np.asarray(inputs[f"bn_b{i}"], np.float32) for i in range(3)]),
    }
    for pre in ("mag", "ph", "wv"):
        shared[f"{pre}w1"] = np.asarray(inputs[f"{pre}_w1"], np.float32)
        shared[f"{pre}b1"] = np.asarray(inputs[f"{pre}_b1"], np.float32)
        shared[f"{pre}w2"] = np.asarray(inputs[f"{pre}_w2"], np.float32)
        shared[f"{pre}b2"] = np.asarray(inputs[f"{pre}_b2"], np.float32)

    lidx = np.arange(4096, dtype=np.float64)
    in_maps = []
    for m in range(NCORES):
        xs = x[m * BS:(m + 1) * BS]
        xpad = np.zeros((BS, 4224), np.float32)
        xpad[:, 64:64 + L] = xs
        xdwt = np.zeros((BS, 4108), np.float32)
        xdwt[:, 7:7 + L] = xs
        xdwt[:, 0:7] = xs[:, 6::-1]
        xdwt[:, 7 + L:] = xs[:, L - 1:L - 8:-1]
        bins = np.arange(256 * m, 256 * m + 256, dtype=np.float64)
        ang = -2.0 * np.pi * np.outer(lidx, bins) / L
        dmat = np.zeros((4096, 512), np.float32)
        dmat[:L, 0:256] = np.cos(ang[:L])
        dmat[:L, 256:512] = np.sin(ang[:L])
        selm = np.zeros((64, 8), np.float32)
        for s in range(BS):
            selm[m * BS + s, s] = 1.0
        im = dict(shared)
        im["sel"] = selm
        im["xpad"] = xpad.astype(BF)
        im["xdwt"] = xdwt
        im["dmat"] = dmat.astype(BF).reshape(32, 128, 512)
        in_maps.append(im)
    return in_maps


def kernel(**inputs) -> np.ndarray:
    from concourse.bass_utils import run_bass_kernel_spmd
    nc = build(1)
    in_maps = pack_inputs(inputs)
    res = run_bass_kernel_spmd(nc, in_maps, list(range(NCORES)))
    out = np.concatenate([np.asarray(res.results[i]["out"]) for i in range(NCORES)], axis=0)
    return out.astype(np.float32)


# revision 18
# speedup vs baseline: 4.3125x; 4.3125x over previous
"""Trainium2 Bass kernel for nn_AdvancedFeatureExtractor (8-core SPMD).

Decomposition (validated against the jax reference in numpy):
  - time conv branch: one shared 128-tap im2col matmul per 512-col chunk
    (all three kernel sizes packed into one (128,84) weight matrix);
    BN batch stats via bn_stats/bn_aggr + cross-core AllGather.
  - freq branch: direct DFT as matmul, frequency-sliced across cores
    (core m computes bins [256m, 256m+256) for ALL 64 samples = band m);
    mag/phase band means exchanged via the same AllGather.
  - wavelet branch: 4-level db4 DWT via block-diagonal strided-conv matmul.
  - fusion MLP: const channels (stats/freq/wavelet feats, constant over L)
    folded into a per-sample bias; only the 84 conv channels go through the
    big per-position matmuls.
"""
import sys
import os
import math

for _p in ("/opt/trn_rl_repo",):
    if _p not in sys.path:
        sys.path.insert(0, _p)

import numpy as np
import ml_dtypes

import concourse.bass as bass
import concourse.mybir as mybir
import concourse.tile as tile
from concourse.bass_types import AP
from concourse.vector_clock import ScopedClock

F32 = mybir.dt.float32
BF16 = mybir.dt.bfloat16
AF = mybir.ActivationFunctionType
OP = mybir.AluOpType
BF = ml_dtypes.bfloat16

L = 4094
B = 64
NCORES = 8
BS = B // NCORES            # 8 samples per core
CH = 512                    # free-dim chunk
NCH_S = 8                   # chunks per sample (last is 510 wide)
LASTW = L - (NCH_S - 1) * CH  # 510
NLOC = BS * L               # per-core BN element count = 32752
NGLOB = B * L               # 262016
BANDW = 256                 # bins per band (= per core)
PI = math.pi

# DWT level geometry
DWT_NIN = [4094, 2051, 1029, 518]
DWT_NOUT = [2051, 1029, 518, 263]
DWT_NPAD = [n + 14 for n in DWT_NIN]
DWT_WE = [(n + 1) // 2 for n in DWT_NPAD]   # even-index width
DWT_WO = [n // 2 for n in DWT_NPAD]         # odd-index width

_DB4_LO = np.array([-0.010597401784997278, 0.032883011666982945, 0.030841381835986965,
                    -0.18703481171888114, -0.02798376941698385, 0.6308807679295904,
                    0.7148465705525415, 0.23037781330885523], np.float32)
_DB4_HI = np.array([-0.23037781330885523, 0.7148465705525415, -0.6308807679295904,
                    -0.02798376941698385, 0.18703481171888114, 0.030841381835986965,
                    0.032883011666982945, -0.010597401784997278], np.float32)
WIN = (32, 64, 128)

_MAXW = 1


def _patched_drain_and_barrier(self, tick_clock, wait_clock):
    """walrus in this container rejects >1 sem wait on a CTRL instruction;
    split the Tile tail-drain waits across single-wait NOPs."""
    nc = self.nc
    probe = nc.sync.nop()
    wait_clock.add_sem_waits(probe.ins, ScopedClock({None: tick_clock.global_clock}))
    si = probe.ins.sync_info
    waits = list(si.on_wait) if si is not None else []
    updates = list(si.on_update) if si is not None else []
    probe.ins.sync_info = mybir.SyncInfo(on_wait=waits[:_MAXW], on_update=updates)
    for i in range(_MAXW, len(waits), _MAXW):
        nop = nc.sync.nop()
        nop.ins.sync_info = mybir.SyncInfo(on_wait=waits[i:i + _MAXW], on_update=[])
    nc.sync.drain()
    nc.all_engine_barrier()
    assert self.sems is not None
    popped = nc._tile_sem_poison_stack.pop()
    assert popped is self._sem_poison
    _sems = list(self.sems.allocated().values())
    _nums = sorted(s.num if hasattr(s, "num") else s for s in _sems)
    for _i in range(0, len(_nums), 8):
        chunk = _nums[_i:_i + 8]
        # only contiguous runs within the chunk
        run = [chunk[0]]
        for n in chunk[1:]:
            if n == run[-1] + 1:
                run.append(n)
            else:
                nc.gpsimd.dma_reset(range(run[0], run[-1] + 1))
                nc.gpsimd.sem_clear(range(run[0], run[-1] + 1))
                run = [n]
        nc.gpsimd.dma_reset(range(run[0], run[-1] + 1))
        nc.gpsimd.sem_clear(range(run[0], run[-1] + 1))
    nc._state.prepend_free_semaphores(_nums)
    for poison_set in nc._tile_sem_poison_stack:
        poison_set.update(_nums)
    nc.all_engine_barrier()


tile.TileContext._drain_and_barrier = _patched_drain_and_barrier


def _split_multi_waits(nc):
    """This walrus build allows only one sem-wait per instruction: hoist extra
    waits onto same-engine NOPs inserted immediately before the instruction."""
    for f in nc.m.functions:
        for bb in f.blocks:
            insts = bb.instructions
            i = 0
            while i < len(insts):
                ins = insts[i]
                si = getattr(ins, "sync_info", None)
                if si is not None and len(si.on_wait) > 1:
                    waits = list(si.on_wait)
                    for j, wsub in enumerate(waits[:-1]):
                        nop = mybir.InstNoOp(
                            name=f"{ins.name}.w{j}", engine=ins.engine,
                            bass_nofuse=True,
                            sync_info=mybir.SyncInfo(on_wait=[wsub], on_update=[]))
                        insts.insert(i, nop)
                        i += 1
                    ins.sync_info = mybir.SyncInfo(
                        on_wait=[waits[-1]], on_update=list(si.on_update))
                i += 1


def _ap(t, offset, dims):
    """Custom access pattern on a tile/param AP `t` (adds t's own offset)."""
    return AP(tensor=t.tensor, offset=t.offset + offset, ap=[list(d) for d in dims])


def build(n_iters: int = 1, phase: float = 5, split_waits: bool = True, bench: bool = False) -> bass.Bass:
    nc = bass.Bass()

    # ---- per-core external inputs ----
    xpad = nc.declare_dram_parameter("xpad", [BS, 4224], BF16, isOutput=False)
    xdwt = nc.declare_dram_parameter("xdwt", [BS, 4108], F32, isOutput=False)
    xt = nc.declare_dram_parameter("xt", [32, 128, 64], BF16, isOutput=False)
    dmat = nc.declare_dram_parameter("dmat", [32, 128, 512], BF16, isOutput=False)
    wall = nc.declare_dram_parameter("wall", [128, 84], BF16, isOutput=False)
    wdwt = nc.declare_dram_parameter("wdwt", [64, 16], F32, isOutput=False)
    w1v = nc.declare_dram_parameter("w1v", [84, 256], BF16, isOutput=False)
    w2p = nc.declare_dram_parameter("w2p", [128, 512], BF16, isOutput=False)
    w1cS = nc.declare_dram_parameter("w1cS", [28, 256], F32, isOutput=False)
    w1cM = nc.declare_dram_parameter("w1cM", [85, 256], F32, isOutput=False)
    w1cP = nc.declare_dram_parameter("w1cP", [85, 256], F32, isOutput=False)
    w1cW = nc.declare_dram_parameter("w1cW", [85, 256], F32, isOutput=False)
    fub1 = nc.declare_dram_parameter("fub1", [128, 2], F32, isOutput=False)
    fub2 = nc.declare_dram_parameter("fub2", [128, 2], F32, isOutput=False)
    stw = nc.declare_dram_parameter("stw", [4, 28], F32, isOutput=False)
    stb = nc.declare_dram_parameter("stb", [28], F32, isOutput=False)
    mlpw = {}
    for pre in ("mag", "ph", "wv"):
        mlpw[pre] = (
            nc.declare_dram_parameter(f"{pre}w1", [8, 85], F32, isOutput=False),
            nc.declare_dram_parameter(f"{pre}b1", [85], F32, isOutput=False),
            nc.declare_dram_parameter(f"{pre}w2", [85, 85], F32, isOutput=False),
            nc.declare_dram_parameter(f"{pre}b2", [85], F32, isOutput=False),
        )
    sel = nc.declare_dram_parameter("sel", [64, 8], F32, isOutput=False)
    eye = nc.declare_dram_parameter("eye", [128, 128], F32, isOutput=False)
    bng = nc.declare_dram_parameter("bng", [84], F32, isOutput=False)
    bnb = nc.declare_dram_parameter("bnb", [84], F32, isOutput=False)
    if bench:
        out_ext = nc.dram_tensor("out_internal", [BS, 256, L], F32)
        dummy = nc.declare_dram_parameter("bench_out", [1, 16], F32, isOutput=True)
    else:
        out_ext = nc.declare_dram_parameter("out", [BS, 256, L], F32, isOutput=True)
        dummy = None

    with tile.TileContext(nc) as tc:
        with tc.tile_pool(name="consts", bufs=1) as cpool, \
             tc.tile_pool(name="ystore", bufs=1) as ypool, \
             tc.tile_pool(name="xck", bufs=4) as xpool, \
             tc.tile_pool(name="dck", bufs=3) as dpool, \
             tc.tile_pool(name="ynp", bufs=3) as ynpool, \
             tc.tile_pool(name="h1p", bufs=4) as h1pool, \
             tc.tile_pool(name="ocp", bufs=4) as ocpool, \
             tc.tile_pool(name="small", bufs=1) as spool, \
             tc.tile_pool(name="dwtsb", bufs=1) as wpool, \
             tc.tile_pool(name="ps", bufs=5, space="PSUM") as ps, \
             tc.tile_pool(name="psdft", bufs=1, space="PSUM") as psdft, \
             tc.tile_pool(name="psmlp", bufs=2, space="PSUM") as psmlp, \
             tc.tile_pool(name="dram", bufs=2, space="DRAM") as dram:

            # ================= constants =================
            wall_sb = cpool.tile([128, 84], BF16)
            nc.gpsimd.dma_start(wall_sb[:], wall[:])
            xt_sb = cpool.tile([128, 2048], BF16)
            for c in range(32):
                nc.gpsimd.dma_start(xt_sb[:, 64 * c:64 * c + 64], xt[c])
            w1v_sb = cpool.tile([84, 256], BF16)
            nc.gpsimd.dma_start(w1v_sb[:], w1v[:])
            w2_sb = cpool.tile([128, 512], BF16)
            nc.gpsimd.dma_start(w2_sb[:], w2p[:])
            w1cS_sb = cpool.tile([28, 256], F32)
            nc.gpsimd.dma_start(w1cS_sb[:], w1cS[:])
            w1cM_sb = cpool.tile([85, 256], F32)
            nc.gpsimd.dma_start(w1cM_sb[:], w1cM[:])
            w1cP_sb = cpool.tile([85, 256], F32)
            nc.gpsimd.dma_start(w1cP_sb[:], w1cP[:])
            w1cW_sb = cpool.tile([85, 256], F32)
            nc.gpsimd.dma_start(w1cW_sb[:], w1cW[:])
            fub1_sb = cpool.tile([128, 2], F32)
            nc.gpsimd.dma_start(fub1_sb[:], fub1[:])
            fub2_sb = cpool.tile([128, 2], F32)
            nc.gpsimd.dma_start(fub2_sb[:], fub2[:])
            stw_sb = cpool.tile([4, 28], F32)
            nc.gpsimd.dma_start(stw_sb[:], stw[:])
            stb_sb = cpool.tile([28, 1], F32)
            nc.gpsimd.dma_start(stb_sb[:], _ap(stb[:], 0, [[1, 28], [1, 1]]))
            mlp_sb = {}
            for pre in ("mag", "ph", "wv"):
                p1, pb1, p2, pb2 = mlpw[pre]
                w1_sb = cpool.tile([8, 85], F32, name=f"{pre}w1sb")
                nc.gpsimd.dma_start(w1_sb[:], p1[:])
                b1_sb = cpool.tile([85, 1], F32, name=f"{pre}b1sb")
                nc.gpsimd.dma_start(b1_sb[:], _ap(pb1[:], 0, [[1, 85], [1, 1]]))
                w2_sb2 = cpool.tile([85, 85], F32, name=f"{pre}w2sb")
                nc.gpsimd.dma_start(w2_sb2[:], p2[:])
                b2_sb = cpool.tile([85, 1], F32, name=f"{pre}b2sb")
                nc.gpsimd.dma_start(b2_sb[:], _ap(pb2[:], 0, [[1, 85], [1, 1]]))
                mlp_sb[pre] = (w1_sb, b1_sb, w2_sb2, b2_sb)
            bng_sb = cpool.tile([84, 1], F32)
            nc.gpsimd.dma_start(bng_sb[:], _ap(bng[:], 0, [[1, 84], [1, 1]]))
            bnb_sb = cpool.tile([84, 1], F32)
            nc.gpsimd.dma_start(bnb_sb[:], _ap(bnb[:], 0, [[1, 84], [1, 1]]))
            wdwt_sb = cpool.tile([64, 16], F32)
            nc.gpsimd.dma_start(wdwt_sb[:], wdwt[:])
            ones8 = cpool.tile([8, 1], F32)
            nc.vector.memset(ones8[:], 1.0)
            sel_sb = cpool.tile([64, 8], F32)
            nc.gpsimd.dma_start(sel_sb[:], sel[:])
            eye_sb = cpool.tile([128, 128], F32)
            nc.gpsimd.dma_start(eye_sb[:], eye[:])

            for it in range(n_iters):
                # ============ conv branch: im2col matmul + bn stats ============
                y_store = ypool.tile([84, NCH_S * BS * CH], BF16)
                bnstat = spool.tile([84, 6 * NCH_S * BS], F32)
                for s in range(BS):
                    for ci in range(NCH_S):
                        idx = s * NCH_S + ci
                        w = LASTW if ci == NCH_S - 1 else CH
                        xc = xpool.tile([128, CH], BF16)
                        nc.sync.dma_start(
                            xc[:], _ap(xpad[:], s * 4224 + ci * CH, [[1, 128], [1, CH]]))
                        yp = ps.tile([84, CH], F32, tag="bigps")
                        nc.tensor.matmul(yp[:], wall_sb[:], xc[:])
                        nc.scalar.copy(y_store[:, idx * CH:(idx + 1) * CH], yp[:])
                        nc.vector.bn_stats(bnstat[:, idx * 6:idx * 6 + 6], yp[:, :w])
                bnmv = spool.tile([84, 2], F32)
                nc.vector.bn_aggr(bnmv[:], bnstat[:])

                if phase < 1.2:
                    continue
                # ============ DFT branch (own 256 bins, all 64 samples) ============
                dacc = psdft.tile([64, 512], F32)
                for c in range(32):
                    dt_ = dpool.tile([128, 512], BF16)
                    nc.sync.dma_start(dt_[:], dmat[c])
                    nc.tensor.matmul(dacc[:], xt_sb[:, c * 64:(c + 1) * 64], dt_[:],
                                     start=(c == 0), stop=(c == 31))
                if phase < 1.3:
                    continue
                bufs8 = [spool.tile([64, 256], F32, name=f"dp{i}") for i in range(8)]
                b1, b2, b3, b4, b5, b6, b7, b8 = [t[:] for t in bufs8]
                nc.vector.tensor_copy(b1, dacc[:, 0:256])   # re -> SBUF
                nc.vector.tensor_copy(b2, dacc[:, 256:512])  # im -> SBUF
                nc.vector.tensor_tensor(b3, b1, b1, OP.mult)
                nc.vector.tensor_tensor(b4, b2, b2, OP.mult)
                nc.vector.tensor_tensor(b5, b3, b4, OP.add)
                nc.scalar.sqrt(b6, b5)
                magm = spool.tile([64, 1], F32)
                nc.vector.tensor_reduce(magm[:], b6, mybir.AxisListType.X, OP.add)
                magmean = spool.tile([64, 1], F32)
                nc.scalar.mul(magmean[:], magm[:], 1.0 / BANDW)
                if phase < 1.6:
                    continue
                # atan2(im, re); arctan LUT domain is [-pi/2, pi/2] so range-reduce
                nc.scalar.activation(b3, b2, AF.Abs)  # |im|
                nc.scalar.activation(b4, b1, AF.Abs)  # |re|
                nc.vector.tensor_tensor(b5, b3, b4, OP.min)
                nc.vector.tensor_tensor(b6, b3, b4, OP.max)
                nc.vector.reciprocal(b7, b6)
                nc.vector.tensor_tensor(b8, b5, b7, OP.mult)
                nc.scalar.activation(b5, b8, AF.Arctan)
                nc.vector.tensor_tensor(b6, b3, b4, OP.is_gt)   # swap flag
                nc.vector.tensor_scalar(b7, b6, -2.0, 1.0, OP.mult, OP.add)
                nc.vector.tensor_tensor(b8, b5, b7, OP.mult)
                nc.vector.scalar_tensor_tensor(b7, b6, PI / 2, b8, OP.mult, OP.add)  # ang0
                nc.vector.tensor_scalar(b3, b1, 0.0, None, OP.is_lt)   # re<0
                nc.vector.tensor_scalar(b4, b3, -2.0, 1.0, OP.mult, OP.add)
                nc.vector.tensor_tensor(b5, b7, b4, OP.mult)
                nc.vector.scalar_tensor_tensor(b6, b3, PI, b5, OP.mult, OP.add)      # ang1
                nc.vector.tensor_scalar(b4, b2, 0.0, None, OP.is_ge)
                nc.vector.tensor_scalar(b5, b4, 2.0, -1.0, OP.mult, OP.add)          # sign(im)
                nc.vector.tensor_tensor(b8, b6, b5, OP.mult)
                phm = spool.tile([64, 1], F32)
                nc.vector.tensor_reduce(phm[:], b8, mybir.AxisListType.X, OP.add)
                phmean = spool.tile([64, 1], F32)
                nc.scalar.mul(phmean[:], phm[:], 1.0 / BANDW)

                if phase < 1.8:
                    continue
                # ============ BN partial sums -> AllGather row ============
                bsum = spool.tile([84, 1], F32)
                nc.scalar.mul(bsum[:], bnmv[:, 0:1], float(NLOC))
                msq_ = spool.tile([84, 1], F32)
                nc.vector.tensor_tensor(msq_[:], bnmv[:, 0:1], bnmv[:, 0:1], OP.mult)
                vps = spool.tile([84, 1], F32)
                nc.vector.tensor_tensor(vps[:], bnmv[:, 1:2], msq_[:], OP.add)
                bsq = spool.tile([84, 1], F32)
                nc.vector.tensor_scalar(bsq[:], vps[:], float(NLOC), None, OP.mult)

                ag_src = dram.tile([1, 296], F32)
                ag_dst = dram.tile([8, 296], F32, addr_space="Shared")
                nc.gpsimd.dma_start(_ap(ag_src[:], 0, [[1, 64], [1, 1]]), magmean[:])
                nc.gpsimd.dma_start(_ap(ag_src[:], 64, [[1, 64], [1, 1]]), phmean[:])
                nc.gpsimd.dma_start(_ap(ag_src[:], 128, [[1, 84], [1, 1]]), bsum[:])
                nc.gpsimd.dma_start(_ap(ag_src[:], 212, [[1, 84], [1, 1]]), bsq[:])

                if phase < 3:
                    continue
                # ============ x-stats + DWT (overlap with collective) ============
                apad0 = wpool.tile([BS, 4108], F32, name="apad0")
                nc.gpsimd.dma_start(apad0[:], xdwt[:])
                xrow = apad0[:, 7:7 + L]
                xjunk = wpool.tile([BS, L], BF16, name="xjunk")
                xsum = spool.tile([BS, 1], F32)
                nc.scalar.activation(xjunk[:], xrow, AF.Copy, accum_out=xsum[:])
                xsqs = spool.tile([BS, 1], F32)
                nc.scalar.activation(xjunk[:], xrow, AF.Square, accum_out=xsqs[:])
                xmax = spool.tile([BS, 1], F32)
                nc.vector.tensor_reduce(xmax[:], xrow, mybir.AxisListType.X, OP.max)
                xmin = spool.tile([BS, 1], F32)
                nc.vector.tensor_reduce(xmin[:], xrow, mybir.AxisListType.X, OP.min)
                xmean = spool.tile([BS, 1], F32)
                nc.scalar.mul(xmean[:], xsum[:], 1.0 / L)
                xmsq = spool.tile([BS, 1], F32)
                nc.vector.tensor_tensor(xmsq[:], xmean[:], xmean[:], OP.mult)
                xu = spool.tile([BS, 1], F32)
                nc.vector.tensor_scalar(xu[:], xmsq[:], -float(L) / (L - 1), None, OP.mult)
                xv_ = spool.tile([BS, 1], F32)
                nc.vector.scalar_tensor_tensor(xv_[:], xsqs[:], 1.0 / (L - 1), xu[:], OP.mult, OP.add)
                xstd = spool.tile([BS, 1], F32)
                nc.scalar.sqrt(xstd[:], xv_[:])
                statdram = dram.tile([4, 8], F32)
                for r, tl in enumerate((xmean, xstd, xmax, xmin)):
                    nc.gpsimd.dma_start(_ap(statdram[:], 8 * r, [[1, 8], [1, 1]]), tl[:])

                # DWT levels
                wf_dram = dram.tile([8, 16], F32)
                apad_cur = apad0  # (8, 4108) level-0 padded input
                for lv in range(4):
                    nin, nout = DWT_NIN[lv], DWT_NOUT[lv]
                    npad, we, wo = DWT_NPAD[lv], DWT_WE[lv], DWT_WO[lv]
                    nrow = BS if lv == 0 else 16
                    # deinterleave into even/odd, bounce through DRAM
                    ae = wpool.tile([nrow, we], F32, name=f"ae{lv}", tag="aet")
                    nc.vector.tensor_copy(
                        ae[:], _ap(apad_cur[:], 0, [[apad_cur.ap[0][0], nrow], [2, we]]))
                    ao = wpool.tile([nrow, wo], F32, name=f"ao{lv}", tag="aot")
                    nc.vector.tensor_copy(
                        ao[:], _ap(apad_cur[:], 1, [[apad_cur.ap[0][0], nrow], [2, wo]]))
                    ae_d = dram.tile([nrow, we], F32, name=f"aed{lv}")
                    nc.gpsimd.dma_start(ae_d[:], ae[:])
                    ao_d = dram.tile([nrow, wo], F32, name=f"aod{lv}")
                    nc.gpsimd.dma_start(ao_d[:], ao[:])
                    xlv = wpool.tile([64, nout], F32, name=f"xlv{lv}", tag="xlv")
                    rstep = we if lv == 0 else 2 * we
                    rstepo = wo if lv == 0 else 2 * wo
                    for t in range(8):
                        srcd = ae_d if t % 2 == 0 else ao_d
                        rst = rstep if t % 2 == 0 else rstepo
                        nc.sync.dma_start(
                            xlv[8 * t:8 * t + 8, :],
                            _ap(srcd[:], t // 2, [[rst, 8], [1, nout]]))
                    # next-level padded buffer (evac target)
                    nch = (nout + CH - 1) // CH
                    if lv < 3:
                        apad_next = wpool.tile([16, DWT_NPAD[lv + 1]], F32, name=f"apad{lv + 1}", tag="apadA" if (lv + 1) % 2 == 0 else "apadB")
                        evtgt = apad_next
                        evoff = 7
                    else:
                        evtgt = wpool.tile([16, nout], F32, name="apadj", tag="apadA")
                        evoff = 0
                    csums = spool.tile([16, 8], F32, name=f"cs{lv}")
                    cabss = spool.tile([16, 8], F32, name=f"ca{lv}")
                    csqs = spool.tile([16, 8], F32, name=f"cq{lv}")
                    for c in range(nch):
                        c0 = c * CH
                        wch = min(CH, nout - c0)
                        dp = ps.tile([16, CH], F32, tag="bigps", name=f"dwtp{lv}")
                        nc.tensor.matmul(dp[:, :wch], wdwt_sb[:], xlv[:, c0:c0 + wch])
                        nc.scalar.activation(evtgt[:, evoff + c0:evoff + c0 + wch],
                                             dp[:, :wch], AF.Copy,
                                             accum_out=csums[:, c:c + 1])
                        junka = wpool.tile([16, CH], BF16, name="junka")
                        nc.scalar.activation(junka[:, :wch], dp[:, :wch], AF.Abs,
                                             accum_out=cabss[:, c:c + 1])
                        junkb = wpool.tile([16, CH], F32, name="junkb")
                        cad = evtgt[:, evoff + c0:evoff + c0 + wch]
                        nc.vector.tensor_tensor(junkb[:, :wch], cad, cad, OP.mult)
                        nc.vector.tensor_reduce(csqs[:, c:c + 1], junkb[:, :wch],
                                                mybir.AxisListType.X, OP.add)
                    if lv < 3:
                        # symmetric pads for next level
                        npn = DWT_NPAD[lv + 1]
                        nc.vector.tensor_copy(
                            apad_next[:, 0:7],
                            _ap(apad_next[:], 13, [[apad_next.ap[0][0], 16], [-1, 7]]))
                        nc.vector.tensor_copy(
                            apad_next[:, 7 + nout:npn],
                            _ap(apad_next[:], 7 + nout - 1,
                                [[apad_next.ap[0][0], 16], [-1, 7]]))
                        apad_cur = apad_next
                    # reduce chunk stats -> level features
                    ctot = spool.tile([16, 1], F32, name=f"ct{lv}")
                    nc.vector.tensor_reduce(ctot[:], csums[:, :nch], mybir.AxisListType.X, OP.add)
                    atot = spool.tile([16, 1], F32, name=f"at{lv}")
                    nc.vector.tensor_reduce(atot[:], cabss[:, :nch], mybir.AxisListType.X, OP.add)
                    qtot = spool.tile([16, 1], F32, name=f"qt{lv}")
                    nc.vector.tensor_reduce(qtot[:], csqs[:, :nch], mybir.AxisListType.X, OP.add)
                    man = spool.tile([16, 1], F32, name=f"man{lv}")
                    nc.scalar.mul(man[:], atot[:], 1.0 / nout)
                    cmean = spool.tile([16, 1], F32, name=f"cm{lv}")
                    nc.scalar.mul(cmean[:], ctot[:], 1.0 / nout)
                    cmsq = spool.tile([16, 1], F32, name=f"cmq{lv}")
                    nc.vector.tensor_tensor(cmsq[:], cmean[:], cmean[:], OP.mult)
                    cvar = spool.tile([16, 1], F32, name=f"cv{lv}")
                    nc.vector.scalar_tensor_tensor(cvar[:], qtot[:], 1.0 / nout, cmsq[:],
                                                   OP.mult, OP.subtract)
                    cstd = spool.tile([16, 1], F32, name=f"cd{lv}")
                    nc.scalar.sqrt(cstd[:], cvar[:])
                    r0 = 2 * (3 - lv)
                    nc.gpsimd.dma_start(_ap(wf_dram[:], 16 * r0, [[1, 16], [1, 1]]), man[:])
                    nc.gpsimd.dma_start(_ap(wf_dram[:], 16 * (r0 + 1), [[1, 16], [1, 1]]), cstd[:])

                if phase < 4:
                    continue
                # ============ collective ============
                nc.gpsimd.collective_compute(
                    "AllGather", OP.bypass,
                    replica_groups=[list(range(NCORES))],
                    ins=[ag_src.opt()], outs=[ag_dst.opt()])

                # ============ post-collective: BN + const features ============
                gath = spool.tile([8, 296], F32)
                nc.gpsimd.dma_start(gath[:], ag_dst[:])
                gsp = psmlp.tile([1, 296], F32, tag="mlpp")
                nc.tensor.matmul(gsp[:], ones8[:], gath[:])
                gsums = spool.tile([1, 296], F32)
                nc.vector.tensor_copy(gsums[:], gsp[:])
                gb_d = dram.tile([1, 296], F32)
                nc.gpsimd.dma_start(gb_d[:], gsums[:])
                gs84 = spool.tile([84, 1], F32)
                nc.gpsimd.dma_start(gs84[:], _ap(gb_d[:], 128, [[1, 84], [1, 1]]))
                gq84 = spool.tile([84, 1], F32)
                nc.gpsimd.dma_start(gq84[:], _ap(gb_d[:], 212, [[1, 84], [1, 1]]))
                bmean = spool.tile([84, 1], F32)
                nc.scalar.mul(bmean[:], gs84[:], 1.0 / NGLOB)
                bmneg = spool.tile([84, 1], F32)
                nc.scalar.mul(bmneg[:], gs84[:], -1.0 / NGLOB)
                bmsq = spool.tile([84, 1], F32)
                nc.vector.tensor_tensor(bmsq[:], bmean[:], bmneg[:], OP.mult)
                bvar = spool.tile([84, 1], F32)
                nc.vector.scalar_tensor_tensor(bvar[:], gq84[:], 1.0 / NGLOB, bmsq[:],
                                               OP.mult, OP.add)
                bve = spool.tile([84, 1], F32)
                nc.vector.tensor_scalar(bve[:], bvar[:], 1e-5, None, OP.add)
                bsd = spool.tile([84, 1], F32)
                nc.scalar.sqrt(bsd[:], bve[:])
                brq = spool.tile([84, 1], F32)
                nc.vector.reciprocal(brq[:], bsd[:])
                bnscale = spool.tile([84, 1], F32)
                nc.vector.tensor_tensor(bnscale[:], brq[:], bng_sb[:], OP.mult)
                bnms = spool.tile([84, 1], F32)
                nc.vector.tensor_tensor(bnms[:], bmneg[:], bnscale[:], OP.mult)
                bnbias = spool.tile([84, 1], F32)
                nc.vector.tensor_tensor(bnbias[:], bnms[:], bnb_sb[:], OP.add)

                statsT = spool.tile([4, 8], F32)
                nc.gpsimd.dma_start(statsT[:], statdram[:])
                wfT = spool.tile([8, 8], F32)
                for s_ in range(8):
                    nc.gpsimd.dma_start(
                        wfT[:, s_:s_ + 1],
                        _ap(wf_dram[:], 2 * s_ + 1, [[16, 8], [1, 1]]))

                cfS = spool.tile([28, 8], F32)
                sfp = psmlp.tile([28, 8], F32, tag="mlpp")
                nc.tensor.matmul(sfp[:], stw_sb[:], statsT[:])
                nc.scalar.activation(cfS[:], sfp[:], AF.Identity, bias=stb_sb[:])
                # band MLPs: all-64-sample layer-1, select own samples via
                # one-hot matmul (selection commutes with bias/relu), then
                # PE-transpose so biases become per-partition.
                cfM = spool.tile([85, 8], F32)
                cfP = spool.tile([85, 8], F32)
                cfW = spool.tile([85, 8], F32)
                for pre, goff, tgt in (("mag", 0, cfM), ("ph", 64, cfP)):
                    w1_sb, b1_sb, w2_sb2, b2_sb = mlp_sb[pre]
                    h64p = psmlp.tile([64, 85], F32, tag="mlpp", name=f"h64p{pre}")
                    nc.tensor.matmul(h64p[:], gath[0:8, goff:goff + 64], w1_sb[:])
                    h64 = spool.tile([64, 85], F32, name=f"h64{pre}")
                    nc.vector.tensor_copy(h64[:], h64p[:])
                    hselp = psmlp.tile([8, 85], F32, tag="mlpp", name=f"hsel{pre}")
                    nc.tensor.matmul(hselp[:], sel_sb[:], h64[:])
                    hsel = spool.tile([8, 85], F32, name=f"hselS{pre}")
                    nc.vector.tensor_copy(hsel[:], hselp[:])
                    htp = psmlp.tile([85, 8], F32, tag="mlpp", name=f"htp{pre}")
                    nc.tensor.transpose(htp[:], hsel[:], eye_sb[0:8, 0:8])
                    ht = spool.tile([85, 8], F32, name=f"ht{pre}")
                    nc.scalar.activation(ht[:], htp[:], AF.Relu, bias=b1_sb[:])
                    op2 = psmlp.tile([85, 8], F32, tag="mlpp", name=f"op{pre}")
                    nc.tensor.matmul(op2[:], w2_sb2[:], ht[:])
                    nc.scalar.activation(tgt[:], op2[:], AF.Identity, bias=b2_sb[:])
                # wavelet MLP (local samples, simple orientation)
                w1_sb, b1_sb, w2_sb2, b2_sb = mlp_sb["wv"]
                hpw = psmlp.tile([85, 8], F32, tag="mlpp", name="hpwv")
                nc.tensor.matmul(hpw[:], w1_sb[:], wfT[:])
                hhw = spool.tile([85, 8], F32, name="hhwv")
                nc.scalar.activation(hhw[:], hpw[:], AF.Relu, bias=b1_sb[:])
                opw = psmlp.tile([85, 8], F32, tag="mlpp", name="opwv")
                nc.tensor.matmul(opw[:], w2_sb2[:], hhw[:])
                nc.scalar.activation(cfW[:], opw[:], AF.Identity, bias=b2_sb[:])
                # const-channel contribution to layer-1 bias
                cbT = []
                for oh in range(2):
                    cbp = psmlp.tile([128, 8], F32, tag="mlpp", name=f"cbp{oh}")
                    sl = slice(oh * 128, oh * 128 + 128)
                    nc.tensor.matmul(cbp[:], w1cS_sb[:, sl], cfS[:], start=True, stop=False)
                    nc.tensor.matmul(cbp[:], w1cM_sb[:, sl], cfM[:], start=False, stop=False)
                    nc.tensor.matmul(cbp[:], w1cP_sb[:, sl], cfP[:], start=False, stop=False)
                    nc.tensor.matmul(cbp[:], w1cW_sb[:, sl], cfW[:], start=False, stop=True)
                    cb = spool.tile([128, 8], F32, name=f"cbT{oh}")
                    nc.scalar.activation(cb[:], cbp[:], AF.Identity,
                                         bias=fub1_sb[:, oh:oh + 1])
                    cbT.append(cb)

                if phase < 5:
                    continue
                # ============ fusion ============
                for s in range(BS):
                    for ci in range(NCH_S):
                        idx = s * NCH_S + ci
                        w = LASTW if ci == NCH_S - 1 else CH
                        yn = ynpool.tile([84, CH], BF16)
                        nc.scalar.activation(yn[:], y_store[:, idx * CH:(idx + 1) * CH],
                                             AF.Relu, bias=bnbias[:], scale=bnscale[:])
                        h1s = []
                        for oh in range(2):
                            hp1 = ps.tile([128, CH], F32, tag="bigps", name=f"hps{oh}")
                            nc.tensor.matmul(hp1[:], w1v_sb[:, oh * 128:(oh + 1) * 128], yn[:])
                            h1 = h1pool.tile([128, CH], BF16, name=f"h1{oh}")
                            if oh == 0:
                                nc.scalar.activation(h1[:], hp1[:], AF.Relu,
                                                     bias=cbT[0][:, s:s + 1])
                            else:
                                nc.vector.tensor_scalar(h1[:], hp1[:], cbT[1][:, s:s + 1],
                                                        0.0, OP.add, OP.max)
                            h1s.append(h1)
                        for of in range(2):
                            op_ = ps.tile([128, CH], F32, tag="bigps", name=f"ops{of}")
                            nc.tensor.matmul(op_[:], w2_sb[:, (0 * 2 + of) * 128:(0 * 2 + of) * 128 + 128],
                                             h1s[0][:], start=True, stop=False)
                            nc.tensor.matmul(op_[:], w2_sb[:, (1 * 2 + of) * 128:(1 * 2 + of) * 128 + 128],
                                             h1s[1][:], start=False, stop=True)
                            oc = ocpool.tile([128, CH], F32, name=f"oc{of}")
                            if of == 0:
                                nc.scalar.activation(oc[:], op_[:], AF.Identity,
                                                     bias=fub2_sb[:, 0:1])
                            else:
                                nc.vector.tensor_scalar(oc[:], op_[:], fub2_sb[:, 1:2],
                                                        None, OP.add)
                            nc.sync.dma_start(
                                _ap(out_ext[:], s * 256 * L + (of * 128) * L + ci * CH,
                                    [[L, 128], [1, w]]),
                                oc[:, :w])
            if bench:
                dnull = spool.tile([1, 16], F32, name="dnull")
                nc.vector.memset(dnull[:], 1.0)
                nc.gpsimd.dma_start(dummy[:], dnull[:])
    if split_waits:
        _split_multi_waits(nc)
    return nc


def pack_inputs(inputs: dict) -> list[dict]:
    x = np.asarray(inputs["x"], np.float32)
    fu_w1 = np.asarray(inputs["fu_w1"], np.float32)
    fu_w2 = np.asarray(inputs["fu_w2"], np.float32)

    # shared (replicated) tensors
    wall = np.zeros((128, 84), np.float32)
    for i, k in enumerate(WIN):
        w = np.asarray(inputs[f"tc_w{i}"], np.float32)[:, 0, :]  # (28, k)
        p0 = 64 - k // 2
        wall[p0:p0 + k, i * 28:(i + 1) * 28] = w.T
    wall = wall.astype(BF)

    lo = _DB4_LO[::-1].copy()
    hi = _DB4_HI[::-1].copy()
    wdwt = np.zeros((64, 16), np.float32)
    for t in range(8):
        for s in range(8):
            wdwt[8 * t + s, 2 * s] = lo[t]
            wdwt[8 * t + s, 2 * s + 1] = hi[t]

    xtfull = np.zeros((4096, 64), np.float32)
    xtfull[:L] = x.T
    xt = xtfull.astype(BF).reshape(32, 128, 64)

    w1v = fu_w1[:84].astype(BF)                        # (84, 256)
    w2p = np.zeros((128, 512), np.float32)
    for kh in range(2):
        for oh in range(2):
            w2p[:, (kh * 2 + oh) * 128:(kh * 2 + oh) * 128 + 128] = \
                fu_w2[kh * 128:(kh + 1) * 128, oh * 128:(oh + 1) * 128]
    w2p = w2p.astype(BF)

    w1cS = fu_w1[84:112].copy()     # sf
    w1cM = fu_w1[112:197].copy()    # mag
    w1cP = fu_w1[197:282].copy()    # ph
    w1cW = fu_w1[282:367].copy()    # wf
    fub1 = np.stack([np.asarray(inputs["fu_b1"], np.float32)[:128],
                     np.asarray(inputs["fu_b1"], np.float32)[128:]], axis=1)
    fub2 = np.stack([np.asarray(inputs["fu_b2"], np.float32)[:128],
                     np.asarray(inputs["fu_b2"], np.float32)[128:]], axis=1)

    shared = {
        "eye": np.eye(128, dtype=np.float32),
        "xt": xt, "wall": wall, "wdwt": wdwt.astype(np.float32),
        "w1v": w1v, "w2p": w2p, "w1cS": w1cS, "w1cM": w1cM, "w1cP": w1cP, "w1cW": w1cW,
        "fub1": fub1, "fub2": fub2,
        "stw": np.asarray(inputs["st_w"], np.float32),
        "stb": np.asarray(inputs["st_b"], np.float32),
        "bng": np.concatenate([np.asarray(inputs[f"bn_g{i}"], np.float32) for i in range(3)]),
        "bnb": np.concatenate([np.asarray(inputs[f"bn_b{i}"], np.float32) for i in range(3)]),
    }
    for pre in ("mag", "ph", "wv"):
        shared[f"{pre}w1"] = np.asarray(inputs[f"{pre}_w1"], np.float32)
        shared[f"{pre}b1"] = np.asarray(inputs[f"{pre}_b1"], np.float32)
        shared[f"{pre}w2"] = np.asarray(inputs[f"{pre}_w2"], np.float32)
        shared[f"{pre}b2"] = np.asarray(inputs[f"{pre}_b2"], np.float32)

    lidx = np.arange(4096, dtype=np.float64)
    in_maps = []
    for m in range(NCORES):
        xs = x[m * BS:(m + 1) * BS]
        xpad = np.zeros((BS, 4224), np.float32)
        xpad[:, 64:64 + L] = xs
        xdwt = np.zeros((BS, 4108), np.float32)
        xdwt[:, 7:7 + L] = xs
        xdwt[:, 0:7] = xs[:, 6::-1]
        xdwt[:, 7 + L:] = xs[:, L - 1:L - 8:-1]
        bins = np.arange(256 * m, 256 * m + 256, dtype=np.float64)
        ang = -2.0 * np.pi * np.outer(lidx, bins) / L
        dmat = np.zeros((4096, 512), np.float32)
        dmat[:L, 0:256] = np.cos(ang[:L])
        dmat[:L, 256:512] = np.sin(ang[:L])
        selm = np.zeros((64, 8), np.float32)
        for s in range(BS):
            selm[m * BS + s, s] = 1.0
        im = dict(shared)
        im["sel"] = selm
        im["xpad"] = xpad.astype(BF)
        im["xdwt"] = xdwt
        im["dmat"] = dmat.astype(BF).reshape(32, 128, 512)
        in_maps.append(im)
    return in_maps


def kernel(**inputs) -> np.ndarray:
    from concourse.bass_utils import run_bass_kernel_spmd
    nc = build(1)
    in_maps = pack_inputs(inputs)
    res = run_bass_kernel_spmd(nc, in_maps, list(range(NCORES)))
    out = np.concatenate([np.asarray(res.results[i]["out"]) for i in range(NCORES)], axis=0)
    return out.astype(np.float32)


# revision 23
# speedup vs baseline: 29.4284x; 6.8240x over previous
"""Trainium2 Bass kernel for nn_AdvancedFeatureExtractor (8-core SPMD).

Decomposition (validated against the jax reference in numpy):
  - time conv branch: one shared 128-tap im2col matmul per 512-col chunk
    (all three kernel sizes packed into one (128,84) weight matrix);
    BN batch stats via bn_stats/bn_aggr + cross-core AllGather.
  - freq branch: direct DFT as matmul, frequency-sliced across cores
    (core m computes bins [256m, 256m+256) for ALL 64 samples = band m);
    mag/phase band means exchanged via the same AllGather.
  - wavelet branch: 4-level db4 DWT via block-diagonal strided-conv matmul.
  - fusion MLP: const channels (stats/freq/wavelet feats, constant over L)
    folded into a per-sample bias; only the 84 conv channels go through the
    big per-position matmuls.
"""
import sys
import os
import math

for _p in ("/opt/trn_rl_repo",):
    if _p not in sys.path:
        sys.path.insert(0, _p)

import numpy as np
import ml_dtypes

import concourse.bass as bass
import concourse.mybir as mybir
import concourse.tile as tile
from concourse.bass_types import AP
from concourse.vector_clock import ScopedClock

F32 = mybir.dt.float32
BF16 = mybir.dt.bfloat16
AF = mybir.ActivationFunctionType
OP = mybir.AluOpType
BF = ml_dtypes.bfloat16

L = 4094
B = 64
NCORES = 8
BS = B // NCORES            # 8 samples per core
CH = 512                    # free-dim chunk
NCH_S = 8                   # chunks per sample (last is 510 wide)
LASTW = L - (NCH_S - 1) * CH  # 510
NLOC = BS * L               # per-core BN element count = 32752
NGLOB = B * L               # 262016
BANDW = 256                 # bins per band (= per core)
PI = math.pi

# DWT level geometry
DWT_NIN = [4094, 2051, 1029, 518]
DWT_NOUT = [2051, 1029, 518, 263]
DWT_NPAD = [n + 14 for n in DWT_NIN]
DWT_WE = [(n + 1) // 2 for n in DWT_NPAD]   # even-index width
DWT_WO = [n // 2 for n in DWT_NPAD]         # odd-index width

_DB4_LO = np.array([-0.010597401784997278, 0.032883011666982945, 0.030841381835986965,
                    -0.18703481171888114, -0.02798376941698385, 0.6308807679295904,
                    0.7148465705525415, 0.23037781330885523], np.float32)
_DB4_HI = np.array([-0.23037781330885523, 0.7148465705525415, -0.6308807679295904,
                    -0.02798376941698385, 0.18703481171888114, 0.030841381835986965,
                    0.032883011666982945, -0.010597401784997278], np.float32)
WIN = (32, 64, 128)

_MAXW = 1


def _patched_drain_and_barrier(self, tick_clock, wait_clock):
    """walrus in this container rejects >1 sem wait on a CTRL instruction;
    split the Tile tail-drain waits across single-wait NOPs."""
    nc = self.nc
    probe = nc.sync.nop()
    wait_clock.add_sem_waits(probe.ins, ScopedClock({None: tick_clock.global_clock}))
    si = probe.ins.sync_info
    waits = list(si.on_wait) if si is not None else []
    updates = list(si.on_update) if si is not None else []
    probe.ins.sync_info = mybir.SyncInfo(on_wait=waits[:_MAXW], on_update=updates)
    for i in range(_MAXW, len(waits), _MAXW):
        nop = nc.sync.nop()
        nop.ins.sync_info = mybir.SyncInfo(on_wait=waits[i:i + _MAXW], on_update=[])
    nc.sync.drain()
    nc.all_engine_barrier()
    assert self.sems is not None
    popped = nc._tile_sem_poison_stack.pop()
    assert popped is self._sem_poison
    _sems = list(self.sems.allocated().values())
    _nums = sorted(s.num if hasattr(s, "num") else s for s in _sems)
    for _i in range(0, len(_nums), 8):
        chunk = _nums[_i:_i + 8]
        # only contiguous runs within the chunk
        run = [chunk[0]]
        for n in chunk[1:]:
            if n == run[-1] + 1:
                run.append(n)
            else:
                nc.gpsimd.dma_reset(range(run[0], run[-1] + 1))
                nc.gpsimd.sem_clear(range(run[0], run[-1] + 1))
                run = [n]
        nc.gpsimd.dma_reset(range(run[0], run[-1] + 1))
        nc.gpsimd.sem_clear(range(run[0], run[-1] + 1))
    nc._state.prepend_free_semaphores(_nums)
    for poison_set in nc._tile_sem_poison_stack:
        poison_set.update(_nums)
    nc.all_engine_barrier()


tile.TileContext._drain_and_barrier = _patched_drain_and_barrier


def _split_multi_waits(nc):
    """This walrus build allows only one sem-wait per instruction: hoist extra
    waits onto same-engine NOPs inserted immediately before the instruction."""
    for f in nc.m.functions:
        for bb in f.blocks:
            insts = bb.instructions
            i = 0
            while i < len(insts):
                ins = insts[i]
                si = getattr(ins, "sync_info", None)
                if si is not None and len(si.on_wait) > 1:
                    waits = list(si.on_wait)
                    for j, wsub in enumerate(waits[:-1]):
                        nop = mybir.InstNoOp(
                            name=f"{ins.name}.w{j}", engine=ins.engine,
                            bass_nofuse=True,
                            sync_info=mybir.SyncInfo(on_wait=[wsub], on_update=[]))
                        insts.insert(i, nop)
                        i += 1
                    ins.sync_info = mybir.SyncInfo(
                        on_wait=[waits[-1]], on_update=list(si.on_update))
                i += 1


def _ap(t, offset, dims):
    """Custom access pattern on a tile/param AP `t` (adds t's own offset)."""
    return AP(tensor=t.tensor, offset=t.offset + offset, ap=[list(d) for d in dims])


def build(n_iters: int = 1, phase: float = 5, split_waits: bool = True, bench: bool = False) -> bass.Bass:
    nc = bass.Bass()

    # ---- per-core external inputs ----
    xpad = nc.declare_dram_parameter("xpad", [BS, 4224], BF16, isOutput=False)
    xdwt = nc.declare_dram_parameter("xdwt", [BS, 4108], F32, isOutput=False)
    xt = nc.declare_dram_parameter("xt", [32, 128, 64], BF16, isOutput=False)
    dmat = nc.declare_dram_parameter("dmat", [32, 128, 512], BF16, isOutput=False)
    wall = nc.declare_dram_parameter("wall", [128, 84], BF16, isOutput=False)
    wallf = nc.declare_dram_parameter("wallf", [128, 84], F32, isOutput=False)
    wdwt = nc.declare_dram_parameter("wdwt", [64, 16], BF16, isOutput=False)
    w1v = nc.declare_dram_parameter("w1v", [84, 256], BF16, isOutput=False)
    w2p = nc.declare_dram_parameter("w2p", [128, 512], BF16, isOutput=False)
    w1cS = nc.declare_dram_parameter("w1cS", [28, 256], F32, isOutput=False)
    w1cM = nc.declare_dram_parameter("w1cM", [85, 256], F32, isOutput=False)
    w1cP = nc.declare_dram_parameter("w1cP", [85, 256], F32, isOutput=False)
    w1cW = nc.declare_dram_parameter("w1cW", [85, 256], F32, isOutput=False)
    fub1 = nc.declare_dram_parameter("fub1", [128, 2], F32, isOutput=False)
    fub2 = nc.declare_dram_parameter("fub2", [128, 2], F32, isOutput=False)
    stw = nc.declare_dram_parameter("stw", [4, 28], F32, isOutput=False)
    stb = nc.declare_dram_parameter("stb", [28], F32, isOutput=False)
    mlpw = {}
    for pre in ("mag", "ph", "wv"):
        mlpw[pre] = (
            nc.declare_dram_parameter(f"{pre}w1", [8, 85], F32, isOutput=False),
            nc.declare_dram_parameter(f"{pre}b1", [85], F32, isOutput=False),
            nc.declare_dram_parameter(f"{pre}w2", [85, 85], F32, isOutput=False),
            nc.declare_dram_parameter(f"{pre}b2", [85], F32, isOutput=False),
        )
    sel = nc.declare_dram_parameter("sel", [64, 8], F32, isOutput=False)
    eye = nc.declare_dram_parameter("eye", [128, 128], F32, isOutput=False)
    bng = nc.declare_dram_parameter("bng", [84], F32, isOutput=False)
    bnb = nc.declare_dram_parameter("bnb", [84], F32, isOutput=False)
    if bench:
        dummy = nc.declare_dram_parameter("bench_out", [1, 16], F32, isOutput=True)
    else:
        out_ext = nc.declare_dram_parameter("out", [BS, 256, L], F32, isOutput=True)
        dummy = None

    with tile.TileContext(nc) as tc:
        with tc.tile_pool(name="consts", bufs=1) as cpool, \
             tc.tile_pool(name="xstore", bufs=1) as xspool, \
             tc.tile_pool(name="dck", bufs=3) as dpool, \
             tc.tile_pool(name="ynp", bufs=4) as ynpool, \
             tc.tile_pool(name="h1p", bufs=6) as h1pool, \
             tc.tile_pool(name="stg", bufs=2) as stgpool, \
             tc.tile_pool(name="small", bufs=1) as spool, \
             tc.tile_pool(name="dwtsb", bufs=1) as wpool, \
             tc.tile_pool(name="ps", bufs=5, space="PSUM") as ps, \
             tc.tile_pool(name="psdft", bufs=1, space="PSUM") as psdft, \
             tc.tile_pool(name="psmlp", bufs=1, space="PSUM") as psmlp, \
             tc.tile_pool(name="dram", bufs=2, space="DRAM") as dram:

            # ================= constants =================
            wall_sb = cpool.tile([128, 84], BF16)
            nc.gpsimd.dma_start(wall_sb[:], wall[:])
            wallf_sb = cpool.tile([128, 84], F32)
            nc.gpsimd.dma_start(wallf_sb[:], wallf[:])
            xt_sb = cpool.tile([128, 2048], BF16)
            for c in range(32):
                nc.sync.dma_start(xt_sb[:, 64 * c:64 * c + 64], xt[c])
            w1v_sb = cpool.tile([84, 256], BF16)
            nc.gpsimd.dma_start(w1v_sb[:], w1v[:])
            w2_sb = cpool.tile([128, 512], BF16)
            nc.gpsimd.dma_start(w2_sb[:], w2p[:])
            w1cS_sb = cpool.tile([28, 256], F32)
            nc.gpsimd.dma_start(w1cS_sb[:], w1cS[:])
            w1cM_sb = cpool.tile([85, 256], F32)
            nc.gpsimd.dma_start(w1cM_sb[:], w1cM[:])
            w1cP_sb = cpool.tile([85, 256], F32)
            nc.gpsimd.dma_start(w1cP_sb[:], w1cP[:])
            w1cW_sb = cpool.tile([85, 256], F32)
            nc.gpsimd.dma_start(w1cW_sb[:], w1cW[:])
            fub1_sb = cpool.tile([128, 2], F32)
            nc.gpsimd.dma_start(fub1_sb[:], fub1[:])
            fub2_sb = cpool.tile([128, 2], F32)
            nc.gpsimd.dma_start(fub2_sb[:], fub2[:])
            stw_sb = cpool.tile([4, 28], F32)
            nc.gpsimd.dma_start(stw_sb[:], stw[:])
            stb_sb = cpool.tile([28, 1], F32)
            nc.gpsimd.dma_start(stb_sb[:], _ap(stb[:], 0, [[1, 28], [1, 1]]))
            mlp_sb = {}
            for pre in ("mag", "ph", "wv"):
                p1, pb1, p2, pb2 = mlpw[pre]
                w1_sb = cpool.tile([8, 85], F32, name=f"{pre}w1sb")
                nc.gpsimd.dma_start(w1_sb[:], p1[:])
                b1_sb = cpool.tile([85, 1], F32, name=f"{pre}b1sb")
                nc.gpsimd.dma_start(b1_sb[:], _ap(pb1[:], 0, [[1, 85], [1, 1]]))
                w2_sb2 = cpool.tile([85, 85], F32, name=f"{pre}w2sb")
                nc.gpsimd.dma_start(w2_sb2[:], p2[:])
                b2_sb = cpool.tile([85, 1], F32, name=f"{pre}b2sb")
                nc.gpsimd.dma_start(b2_sb[:], _ap(pb2[:], 0, [[1, 85], [1, 1]]))
                mlp_sb[pre] = (w1_sb, b1_sb, w2_sb2, b2_sb)
            bng_sb = cpool.tile([84, 1], F32)
            nc.gpsimd.dma_start(bng_sb[:], _ap(bng[:], 0, [[1, 84], [1, 1]]))
            bnb_sb = cpool.tile([84, 1], F32)
            nc.gpsimd.dma_start(bnb_sb[:], _ap(bnb[:], 0, [[1, 84], [1, 1]]))
            wdwt_sb = cpool.tile([64, 16], BF16)
            nc.gpsimd.dma_start(wdwt_sb[:], wdwt[:])
            ones8 = cpool.tile([8, 1], F32)
            nc.vector.memset(ones8[:], 1.0)
            sel_sb = cpool.tile([64, 8], F32)
            nc.gpsimd.dma_start(sel_sb[:], sel[:])
            eye_sb = cpool.tile([128, 128], F32)
            nc.gpsimd.dma_start(eye_sb[:], eye[:])

            if bench:
                out_ext = dram.tile([BS, 256, L], F32, name="out_bench", bufs=1)

            for it in range(n_iters):
                # ============ conv branch: im2col matmul + bn stats ============
                # X[p, s*4224 + j] = x[s, j + p - 64]; resident for reuse in fusion
                x_all = xspool.tile([128, BS * 4096], BF16)
                for s in range(BS):
                    nc.sync.dma_start(
                        x_all[:, s * 4096:(s + 1) * 4096],
                        _ap(xpad[:], s * 4224, [[1, 128], [1, 4096]]))
                bnstat = spool.tile([84, 6 * NCH_S * BS], F32)
                for s in range(BS):
                    for ci in range(NCH_S):
                        idx = s * NCH_S + ci
                        w = LASTW if ci == NCH_S - 1 else CH
                        yp = ps.tile([84, CH], F32, tag="bigps")
                        nc.tensor.matmul(yp[:], wall_sb[:],
                                         x_all[:, s * 4096 + ci * CH:s * 4096 + (ci + 1) * CH])
                        nc.vector.bn_stats(bnstat[:, idx * 6:idx * 6 + 6], yp[:, :w])
                bnmv = spool.tile([84, 2], F32)
                nc.vector.bn_aggr(bnmv[:], bnstat[:])

                if phase < 1.2:
                    continue
                # ============ DFT branch (own 256 bins, all 64 samples) ============
                dacc = psdft.tile([64, 512], F32)
                for c in range(32):
                    dt_ = dpool.tile([128, 512], BF16)
                    nc.sync.dma_start(dt_[:], dmat[c])
                    nc.tensor.matmul(dacc[:], xt_sb[:, c * 64:(c + 1) * 64], dt_[:],
                                     start=(c == 0), stop=(c == 31))
                if phase < 1.3:
                    continue
                bufs8 = [spool.tile([64, 256], F32, name=f"dp{i}") for i in range(8)]
                b1, b2, b3, b4, b5, b6, b7, b8 = [t[:] for t in bufs8]
                nc.vector.tensor_copy(b1, dacc[:, 0:256])   # re -> SBUF
                nc.vector.tensor_copy(b2, dacc[:, 256:512])  # im -> SBUF
                nc.vector.tensor_tensor(b3, b1, b1, OP.mult)
                nc.vector.tensor_tensor(b4, b2, b2, OP.mult)
                nc.vector.tensor_tensor(b5, b3, b4, OP.add)
                nc.scalar.sqrt(b6, b5)
                magm = spool.tile([64, 1], F32)
                nc.vector.tensor_reduce(magm[:], b6, mybir.AxisListType.X, OP.add)
                magmean = spool.tile([64, 1], F32)
                nc.scalar.mul(magmean[:], magm[:], 1.0 / BANDW)
                if phase < 1.6:
                    continue
                # atan2(im, re); arctan LUT domain is [-pi/2, pi/2] so range-reduce
                nc.scalar.activation(b3, b2, AF.Abs)  # |im|
                nc.scalar.activation(b4, b1, AF.Abs)  # |re|
                nc.vector.tensor_tensor(b5, b3, b4, OP.min)
                nc.vector.tensor_tensor(b6, b3, b4, OP.max)
                nc.vector.reciprocal(b7, b6)
                nc.vector.tensor_tensor(b8, b5, b7, OP.mult)
                nc.scalar.activation(b5, b8, AF.Arctan)
                nc.vector.tensor_tensor(b6, b3, b4, OP.is_gt)   # swap flag
                nc.vector.tensor_scalar(b7, b6, -2.0, 1.0, OP.mult, OP.add)
                nc.vector.tensor_tensor(b8, b5, b7, OP.mult)
                nc.vector.scalar_tensor_tensor(b7, b6, PI / 2, b8, OP.mult, OP.add)  # ang0
                nc.vector.tensor_scalar(b3, b1, 0.0, None, OP.is_lt)   # re<0
                nc.vector.tensor_scalar(b4, b3, -2.0, 1.0, OP.mult, OP.add)
                nc.vector.tensor_tensor(b5, b7, b4, OP.mult)
                nc.vector.scalar_tensor_tensor(b6, b3, PI, b5, OP.mult, OP.add)      # ang1
                nc.vector.tensor_scalar(b4, b2, 0.0, None, OP.is_ge)
                nc.vector.tensor_scalar(b5, b4, 2.0, -1.0, OP.mult, OP.add)          # sign(im)
                nc.vector.tensor_tensor(b8, b6, b5, OP.mult)
                phm = spool.tile([64, 1], F32)
                nc.vector.tensor_reduce(phm[:], b8, mybir.AxisListType.X, OP.add)
                phmean = spool.tile([64, 1], F32)
                nc.scalar.mul(phmean[:], phm[:], 1.0 / BANDW)

                if phase < 1.8:
                    continue
                # ============ BN partial sums -> AllGather row ============
                bsum = spool.tile([84, 1], F32)
                nc.scalar.mul(bsum[:], bnmv[:, 0:1], float(NLOC))
                msq_ = spool.tile([84, 1], F32)
                nc.vector.tensor_tensor(msq_[:], bnmv[:, 0:1], bnmv[:, 0:1], OP.mult)
                vps = spool.tile([84, 1], F32)
                nc.vector.tensor_tensor(vps[:], bnmv[:, 1:2], msq_[:], OP.add)
                bsq = spool.tile([84, 1], F32)
                nc.vector.tensor_scalar(bsq[:], vps[:], float(NLOC), None, OP.mult)

                ag_src = dram.tile([1, 296], F32)
                ag_dst = dram.tile([8, 296], F32, addr_space="Shared")
                nc.gpsimd.dma_start(_ap(ag_src[:], 0, [[1, 64], [1, 1]]), magmean[:])
                nc.gpsimd.dma_start(_ap(ag_src[:], 64, [[1, 64], [1, 1]]), phmean[:])
                nc.gpsimd.dma_start(_ap(ag_src[:], 128, [[1, 84], [1, 1]]), bsum[:])
                nc.gpsimd.dma_start(_ap(ag_src[:], 212, [[1, 84], [1, 1]]), bsq[:])

                # ============ x-stats + DWT (overlap with collective) ============
                apad0 = wpool.tile([BS, 4108], F32, name="apad0")
                nc.gpsimd.dma_start(apad0[:], xdwt[:])
                xrow = apad0[:, 7:7 + L]
                xjunk = wpool.tile([BS, L], BF16, name="xjunk")
                xsum = spool.tile([BS, 1], F32)
                nc.scalar.activation(xjunk[:], xrow, AF.Copy, accum_out=xsum[:])
                xsqs = spool.tile([BS, 1], F32)
                nc.scalar.activation(xjunk[:], xrow, AF.Square, accum_out=xsqs[:])
                xmax = spool.tile([BS, 1], F32)
                nc.vector.tensor_reduce(xmax[:], xrow, mybir.AxisListType.X, OP.max)
                xmin = spool.tile([BS, 1], F32)
                nc.vector.tensor_reduce(xmin[:], xrow, mybir.AxisListType.X, OP.min)
                xmean = spool.tile([BS, 1], F32)
                nc.scalar.mul(xmean[:], xsum[:], 1.0 / L)
                xmsq = spool.tile([BS, 1], F32)
                nc.vector.tensor_tensor(xmsq[:], xmean[:], xmean[:], OP.mult)
                xu = spool.tile([BS, 1], F32)
                nc.vector.tensor_scalar(xu[:], xmsq[:], -float(L) / (L - 1), None, OP.mult)
                xv_ = spool.tile([BS, 1], F32)
                nc.vector.scalar_tensor_tensor(xv_[:], xsqs[:], 1.0 / (L - 1), xu[:], OP.mult, OP.add)
                xstd = spool.tile([BS, 1], F32)
                nc.scalar.sqrt(xstd[:], xv_[:])
                statdram = dram.tile([4, 8], F32)
                for r, tl in enumerate((xmean, xstd, xmax, xmin)):
                    nc.gpsimd.dma_start(_ap(statdram[:], 8 * r, [[1, 8], [1, 1]]), tl[:])

                # DWT levels
                wf_dram = dram.tile([8, 16], F32)
                apad_cur = apad0  # (8, 4108) level-0 padded input
                for lv in range(4):
                    nin, nout = DWT_NIN[lv], DWT_NOUT[lv]
                    npad, we, wo = DWT_NPAD[lv], DWT_WE[lv], DWT_WO[lv]
                    nrow = BS if lv == 0 else 16
                    # deinterleave into even/odd, bounce through DRAM
                    ae = wpool.tile([nrow, we], BF16, name=f"ae{lv}", tag="aet")
                    nc.scalar.copy(
                        ae[:], _ap(apad_cur[:], 0, [[apad_cur.ap[0][0], nrow], [2, we]]))
                    ao = wpool.tile([nrow, wo], BF16, name=f"ao{lv}", tag="aot")
                    nc.scalar.copy(
                        ao[:], _ap(apad_cur[:], 1, [[apad_cur.ap[0][0], nrow], [2, wo]]))
                    ae_d = dram.tile([nrow, we], BF16, name=f"aed{lv}")
                    nc.gpsimd.dma_start(ae_d[:], ae[:])
                    ao_d = dram.tile([nrow, wo], BF16, name=f"aod{lv}")
                    nc.gpsimd.dma_start(ao_d[:], ao[:])
                    xlv = wpool.tile([64, nout], BF16, name=f"xlv{lv}", tag="xlv")
                    rstep = we if lv == 0 else 2 * we
                    rstepo = wo if lv == 0 else 2 * wo
                    for t in range(8):
                        srcd = ae_d if t % 2 == 0 else ao_d
                        rst = rstep if t % 2 == 0 else rstepo
                        eng = nc.sync if t % 4 < 2 else nc.gpsimd
                        eng.dma_start(
                            xlv[8 * t:8 * t + 8, :],
                            _ap(srcd[:], t // 2, [[rst, 8], [1, nout]]))
                    # next-level padded buffer (evac target)
                    nch = (nout + CH - 1) // CH
                    if lv < 3:
                        apad_next = wpool.tile([16, DWT_NPAD[lv + 1]], F32, name=f"apad{lv + 1}", tag="apadA" if (lv + 1) % 2 == 0 else "apadB")
                        evtgt = apad_next
                        evoff = 7
                    else:
                        evtgt = wpool.tile([16, nout], F32, name="apadj", tag="apadA")
                        evoff = 0
                    csums = spool.tile([16, 8], F32, name=f"cs{lv}")
                    cabss = spool.tile([16, 8], F32, name=f"ca{lv}")
                    csqs = spool.tile([16, 8], F32, name=f"cq{lv}")
                    for c in range(nch):
                        c0 = c * CH
                        wch = min(CH, nout - c0)
                        dp = ps.tile([16, CH], F32, tag="bigps", name=f"dwtp{lv}")
                        nc.tensor.matmul(dp[:, :wch], wdwt_sb[:], xlv[:, c0:c0 + wch])
                        nc.scalar.activation(evtgt[:, evoff + c0:evoff + c0 + wch],
                                             dp[:, :wch], AF.Copy,
                                             accum_out=csums[:, c:c + 1])
                        junka = wpool.tile([16, CH], BF16, name="junka")
                        nc.scalar.activation(junka[:, :wch], dp[:, :wch], AF.Abs,
                                             accum_out=cabss[:, c:c + 1])
                        junkb = wpool.tile([16, CH], F32, name="junkb")
                        cad = evtgt[:, evoff + c0:evoff + c0 + wch]
                        nc.vector.tensor_tensor(junkb[:, :wch], cad, cad, OP.mult)
                        nc.vector.tensor_reduce(csqs[:, c:c + 1], junkb[:, :wch],
                                                mybir.AxisListType.X, OP.add)
                    if lv < 3:
                        # symmetric pads for next level
                        npn = DWT_NPAD[lv + 1]
                        nc.vector.tensor_copy(
                            apad_next[:, 0:7],
                            _ap(apad_next[:], 13, [[apad_next.ap[0][0], 16], [-1, 7]]))
                        nc.vector.tensor_copy(
                            apad_next[:, 7 + nout:npn],
                            _ap(apad_next[:], 7 + nout - 1,
                                [[apad_next.ap[0][0], 16], [-1, 7]]))
                        apad_cur = apad_next
                    # reduce chunk stats -> level features
                    ctot = spool.tile([16, 1], F32, name=f"ct{lv}")
                    nc.vector.tensor_reduce(ctot[:], csums[:, :nch], mybir.AxisListType.X, OP.add)
                    atot = spool.tile([16, 1], F32, name=f"at{lv}")
                    nc.vector.tensor_reduce(atot[:], cabss[:, :nch], mybir.AxisListType.X, OP.add)
                    qtot = spool.tile([16, 1], F32, name=f"qt{lv}")
                    nc.vector.tensor_reduce(qtot[:], csqs[:, :nch], mybir.AxisListType.X, OP.add)
                    man = spool.tile([16, 1], F32, name=f"man{lv}")
                    nc.scalar.mul(man[:], atot[:], 1.0 / nout)
                    cmean = spool.tile([16, 1], F32, name=f"cm{lv}")
                    nc.scalar.mul(cmean[:], ctot[:], 1.0 / nout)
                    cmsq = spool.tile([16, 1], F32, name=f"cmq{lv}")
                    nc.vector.tensor_tensor(cmsq[:], cmean[:], cmean[:], OP.mult)
                    cvar = spool.tile([16, 1], F32, name=f"cv{lv}")
                    nc.vector.scalar_tensor_tensor(cvar[:], qtot[:], 1.0 / nout, cmsq[:],
                                                   OP.mult, OP.subtract)
                    cstd = spool.tile([16, 1], F32, name=f"cd{lv}")
                    nc.scalar.sqrt(cstd[:], cvar[:])
                    r0 = 2 * (3 - lv)
                    nc.gpsimd.dma_start(_ap(wf_dram[:], 16 * r0, [[1, 16], [1, 1]]), man[:])
                    nc.gpsimd.dma_start(_ap(wf_dram[:], 16 * (r0 + 1), [[1, 16], [1, 1]]), cstd[:])

                if phase < 4:
                    continue
                # ============ collective ============
                nc.gpsimd.collective_compute(
                    "AllGather", OP.bypass,
                    replica_groups=[list(range(NCORES))],
                    ins=[ag_src.opt()], outs=[ag_dst.opt()])

                # ============ post-collective: BN + const features ============
                gath = spool.tile([8, 296], F32)
                nc.gpsimd.dma_start(gath[:], ag_dst[:])
                gsp = psmlp.tile([1, 296], F32, tag="mlpp")
                nc.tensor.matmul(gsp[:], ones8[:], gath[:])
                gsums = spool.tile([1, 296], F32)
                nc.vector.tensor_copy(gsums[:], gsp[:])
                gb_d = dram.tile([1, 296], F32)
                nc.gpsimd.dma_start(gb_d[:], gsums[:])
                gs84 = spool.tile([84, 1], F32)
                nc.gpsimd.dma_start(gs84[:], _ap(gb_d[:], 128, [[1, 84], [1, 1]]))
                gq84 = spool.tile([84, 1], F32)
                nc.gpsimd.dma_start(gq84[:], _ap(gb_d[:], 212, [[1, 84], [1, 1]]))
                bmean = spool.tile([84, 1], F32)
                nc.scalar.mul(bmean[:], gs84[:], 1.0 / NGLOB)
                bmneg = spool.tile([84, 1], F32)
                nc.scalar.mul(bmneg[:], gs84[:], -1.0 / NGLOB)
                bmsq = spool.tile([84, 1], F32)
                nc.vector.tensor_tensor(bmsq[:], bmean[:], bmneg[:], OP.mult)
                bvar = spool.tile([84, 1], F32)
                nc.vector.scalar_tensor_tensor(bvar[:], gq84[:], 1.0 / NGLOB, bmsq[:],
                                               OP.mult, OP.add)
                bve = spool.tile([84, 1], F32)
                nc.vector.tensor_scalar(bve[:], bvar[:], 1e-5, None, OP.add)
                bsd = spool.tile([84, 1], F32)
                nc.scalar.sqrt(bsd[:], bve[:])
                brq = spool.tile([84, 1], F32)
                nc.vector.reciprocal(brq[:], bsd[:])
                bnscale = spool.tile([84, 1], F32)
                nc.vector.tensor_tensor(bnscale[:], brq[:], bng_sb[:], OP.mult)
                bnms = spool.tile([84, 1], F32)
                nc.vector.tensor_tensor(bnms[:], bmneg[:], bnscale[:], OP.mult)
                bnbias = spool.tile([84, 1], F32)
                nc.vector.tensor_tensor(bnbias[:], bnms[:], bnb_sb[:], OP.add)

                statsT = spool.tile([4, 8], F32)
                nc.gpsimd.dma_start(statsT[:], statdram[:])
                wfT = spool.tile([8, 8], F32)
                for s_ in range(8):
                    nc.gpsimd.dma_start(
                        wfT[:, s_:s_ + 1],
                        _ap(wf_dram[:], 2 * s_ + 1, [[16, 8], [1, 1]]))

                cfS = spool.tile([28, 8], F32)
                sfp = psmlp.tile([28, 8], F32, tag="mlpp")
                nc.tensor.matmul(sfp[:], stw_sb[:], statsT[:])
                nc.scalar.activation(cfS[:], sfp[:], AF.Identity, bias=stb_sb[:])
                # fold BN scale into conv weights: W_bn = W * diag(bnscale)
                zdiag = spool.tile([84, 84], F32, name="zdiag")
                nc.vector.memset(zdiag[:], 0.0)
                diag_d = dram.tile([84, 84], F32, name="diagd")
                nc.gpsimd.dma_start(diag_d[:], zdiag[:])
                nc.gpsimd.dma_start(_ap(diag_d[:], 0, [[85, 84], [1, 1]]), bnscale[:])
                diag_sb = spool.tile([84, 84], F32, name="diagsb")
                nc.gpsimd.dma_start(diag_sb[:], diag_d[:])
                wtp = psmlp.tile([84, 128], F32, tag="mlpp", name="wtp")
                nc.tensor.transpose(wtp[:], wallf_sb[:], eye_sb[:])
                wt_sb = spool.tile([84, 128], F32, name="wtsb")
                nc.vector.tensor_copy(wt_sb[:], wtp[:])
                wbnp = psmlp.tile([128, 84], F32, tag="mlpp", name="wbnp")
                nc.tensor.matmul(wbnp[:], wt_sb[:], diag_sb[:])
                wbn_sb = spool.tile([128, 84], BF16, name="wbnsb")
                nc.scalar.copy(wbn_sb[:], wbnp[:])

                # band MLPs: all-64-sample layer-1, select own samples via
                # one-hot matmul (selection commutes with bias/relu), then
                # PE-transpose so biases become per-partition.
                cfM = spool.tile([85, 8], F32)
                cfP = spool.tile([85, 8], F32)
                cfW = spool.tile([85, 8], F32)
                for pre, goff, tgt in (("mag", 0, cfM), ("ph", 64, cfP)):
                    w1_sb, b1_sb, w2_sb2, b2_sb = mlp_sb[pre]
                    h64p = psmlp.tile([64, 85], F32, tag="mlpp", name=f"h64p{pre}")
                    nc.tensor.matmul(h64p[:], gath[0:8, goff:goff + 64], w1_sb[:])
                    h64 = spool.tile([64, 85], F32, name=f"h64{pre}")
                    nc.vector.tensor_copy(h64[:], h64p[:])
                    hselp = psmlp.tile([8, 85], F32, tag="mlpp", name=f"hsel{pre}")
                    nc.tensor.matmul(hselp[:], sel_sb[:], h64[:])
                    hsel = spool.tile([8, 85], F32, name=f"hselS{pre}")
                    nc.vector.tensor_copy(hsel[:], hselp[:])
                    htp = psmlp.tile([85, 8], F32, tag="mlpp", name=f"htp{pre}")
                    nc.tensor.transpose(htp[:], hsel[:], eye_sb[0:8, 0:8])
                    ht = spool.tile([85, 8], F32, name=f"ht{pre}")
                    nc.scalar.activation(ht[:], htp[:], AF.Relu, bias=b1_sb[:])
                    op2 = psmlp.tile([85, 8], F32, tag="mlpp", name=f"op{pre}")
                    nc.tensor.matmul(op2[:], w2_sb2[:], ht[:])
                    nc.scalar.activation(tgt[:], op2[:], AF.Identity, bias=b2_sb[:])
                # wavelet MLP (local samples, simple orientation)
                w1_sb, b1_sb, w2_sb2, b2_sb = mlp_sb["wv"]
                hpw = psmlp.tile([85, 8], F32, tag="mlpp", name="hpwv")
                nc.tensor.matmul(hpw[:], w1_sb[:], wfT[:])
                hhw = spool.tile([85, 8], F32, name="hhwv")
                nc.scalar.activation(hhw[:], hpw[:], AF.Relu, bias=b1_sb[:])
                opw = psmlp.tile([85, 8], F32, tag="mlpp", name="opwv")
                nc.tensor.matmul(opw[:], w2_sb2[:], hhw[:])
                nc.scalar.activation(cfW[:], opw[:], AF.Identity, bias=b2_sb[:])
                # const-channel contribution to layer-1 bias
                cbT = []
                for oh in range(2):
                    cbp = psmlp.tile([128, 8], F32, tag="mlpp", name=f"cbp{oh}")
                    sl = slice(oh * 128, oh * 128 + 128)
                    nc.tensor.matmul(cbp[:], w1cS_sb[:, sl], cfS[:], start=True, stop=False)
                    nc.tensor.matmul(cbp[:], w1cM_sb[:, sl], cfM[:], start=False, stop=False)
                    nc.tensor.matmul(cbp[:], w1cP_sb[:, sl], cfP[:], start=False, stop=False)
                    nc.tensor.matmul(cbp[:], w1cW_sb[:, sl], cfW[:], start=False, stop=True)
                    cb = spool.tile([128, 8], F32, name=f"cbT{oh}")
                    nc.scalar.activation(cb[:], cbp[:], AF.Identity,
                                         bias=fub1_sb[:, oh:oh + 1])
                    cbT.append(cb)

                if phase < 5:
                    continue
                # ============ fusion ============
                HB = 2048  # staged output columns per DMA
                for s in range(BS):
                    for half in range(2):
                        st0 = stgpool.tile([128, HB], F32, name="st0")
                        st1 = stgpool.tile([128, HB], F32, name="st1")
                        for ci4 in range(4):
                            ci = half * 4 + ci4
                            zp = psmlp.tile([84, CH], F32, tag="zpp", name="zp")
                            nc.tensor.matmul(
                                zp[:], wbn_sb[:],
                                x_all[:, s * 4096 + ci * CH:s * 4096 + (ci + 1) * CH])
                            yn = ynpool.tile([84, CH], BF16)
                            nc.scalar.activation(yn[:], zp[:], AF.Relu, bias=bnbias[:])
                            h1s = []
                            for oh in range(2):
                                hp1 = ps.tile([128, CH], F32, tag="bigps", name=f"hps{oh}")
                                nc.tensor.matmul(hp1[:], w1v_sb[:, oh * 128:(oh + 1) * 128], yn[:])
                                h1 = h1pool.tile([128, CH], BF16, name=f"h1{oh}")
                                if oh == 0:
                                    nc.scalar.activation(h1[:], hp1[:], AF.Relu,
                                                         bias=cbT[0][:, s:s + 1])
                                else:
                                    nc.vector.tensor_scalar(h1[:], hp1[:], cbT[1][:, s:s + 1],
                                                            0.0, OP.add, OP.max)
                                h1s.append(h1)
                            for of in range(2):
                                op_ = ps.tile([128, CH], F32, tag="bigps", name=f"ops{of}")
                                nc.tensor.matmul(op_[:], w2_sb[:, (0 * 2 + of) * 128:(0 * 2 + of) * 128 + 128],
                                                 h1s[0][:], start=True, stop=False)
                                nc.tensor.matmul(op_[:], w2_sb[:, (1 * 2 + of) * 128:(1 * 2 + of) * 128 + 128],
                                                 h1s[1][:], start=False, stop=True)
                                tgt = (st0 if of == 0 else st1)[:, ci4 * CH:(ci4 + 1) * CH]
                                nc.vector.tensor_scalar(tgt, op_[:], fub2_sb[:, of:of + 1],
                                                        None, OP.add)
                        w2c = HB if half == 0 else (L - HB)
                        for of, st in ((0, st0), (1, st1)):
                            nc.sync.dma_start(
                                _ap(out_ext[:], s * 256 * L + (of * 128) * L + half * HB,
                                    [[L, 128], [1, w2c]]),
                                st[:, :w2c])
            if bench:
                dnull = spool.tile([1, 16], F32, name="dnull")
                nc.gpsimd.dma_start(dnull[:], _ap(out_ext[:], 0, [[16, 1], [1, 16]]))
                nc.gpsimd.dma_start(dummy[:], dnull[:])
    if split_waits:
        _split_multi_waits(nc)
    return nc


def pack_inputs(inputs: dict) -> list[dict]:
    x = np.asarray(inputs["x"], np.float32)
    fu_w1 = np.asarray(inputs["fu_w1"], np.float32)
    fu_w2 = np.asarray(inputs["fu_w2"], np.float32)

    # shared (replicated) tensors
    wall = np.zeros((128, 84), np.float32)
    for i, k in enumerate(WIN):
        w = np.asarray(inputs[f"tc_w{i}"], np.float32)[:, 0, :]  # (28, k)
        p0 = 64 - k // 2
        wall[p0:p0 + k, i * 28:(i + 1) * 28] = w.T
    wallf32 = wall.copy()
    wall = wall.astype(BF)

    lo = _DB4_LO[::-1].copy()
    hi = _DB4_HI[::-1].copy()
    wdwt = np.zeros((64, 16), np.float32)
    for t in range(8):
        for s in range(8):
            wdwt[8 * t + s, 2 * s] = lo[t]
            wdwt[8 * t + s, 2 * s + 1] = hi[t]

    xtfull = np.zeros((4096, 64), np.float32)
    xtfull[:L] = x.T
    xt = xtfull.astype(BF).reshape(32, 128, 64)

    w1v = fu_w1[:84].astype(BF)                        # (84, 256)
    w2p = np.zeros((128, 512), np.float32)
    for kh in range(2):
        for oh in range(2):
            w2p[:, (kh * 2 + oh) * 128:(kh * 2 + oh) * 128 + 128] = \
                fu_w2[kh * 128:(kh + 1) * 128, oh * 128:(oh + 1) * 128]
    w2p = w2p.astype(BF)

    w1cS = fu_w1[84:112].copy()     # sf
    w1cM = fu_w1[112:197].copy()    # mag
    w1cP = fu_w1[197:282].copy()    # ph
    w1cW = fu_w1[282:367].copy()    # wf
    fub1 = np.stack([np.asarray(inputs["fu_b1"], np.float32)[:128],
                     np.asarray(inputs["fu_b1"], np.float32)[128:]], axis=1)
    fub2 = np.stack([np.asarray(inputs["fu_b2"], np.float32)[:128],
                     np.asarray(inputs["fu_b2"], np.float32)[128:]], axis=1)

    shared = {
        "eye": np.eye(128, dtype=np.float32),
        "xt": xt, "wall": wall, "wallf": wallf32, "wdwt": wdwt.astype(BF),
        "w1v": w1v, "w2p": w2p, "w1cS": w1cS, "w1cM": w1cM, "w1cP": w1cP, "w1cW": w1cW,
        "fub1": fub1, "fub2": fub2,
        "stw": np.asarray(inputs["st_w"], np.float32),
        "stb": np.asarray(inputs["st_b"], np.float32),
        "bng": np.concatenate([np.asarray(inputs[f"bn_g{i}"], np.float32) for i in range(3)]),
        "bnb": np.concatenate([np.asarray(inputs[f"bn_b{i}"], np.float32) for i in range(3)]),
    }
    for pre in ("mag", "ph", "wv"):
        shared[f"{pre}w1"] = np.asarray(inputs[f"{pre}_w1"], np.float32)
        shared[f"{pre}b1"] = np.asarray(inputs[f"{pre}_b1"], np.float32)
        shared[f"{pre}w2"] = np.asarray(inputs[f"{pre}_w2"], np.float32)
        shared[f"{pre}b2"] = np.asarray(inputs[f"{pre}_b2"], np.float32)

    lidx = np.arange(4096, dtype=np.float64)
    in_maps = []
    for m in range(NCORES):
        xs = x[m * BS:(m + 1) * BS]
        xpad = np.zeros((BS, 4224), np.float32)
        xpad[:, 64:64 + L] = xs
        xdwt = np.zeros((BS, 4108), np.float32)
        xdwt[:, 7:7 + L] = xs
        xdwt[:, 0:7] = xs[:, 6::-1]
        xdwt[:, 7 + L:] = xs[:, L - 1:L - 8:-1]
        bins = np.arange(256 * m, 256 * m + 256, dtype=np.float64)
        ang = -2.0 * np.pi * np.outer(lidx, bins) / L
        dmat = np.zeros((4096, 512), np.float32)
        dmat[:L, 0:256] = np.cos(ang[:L])
        dmat[:L, 256:512] = np.sin(ang[:L])
        selm = np.zeros((64, 8), np.float32)
        for s in range(BS):
            selm[m * BS + s, s] = 1.0
        im = dict(shared)
        im["sel"] = selm
        im["xpad"] = xpad.astype(BF)
        im["xdwt"] = xdwt
        im["dmat"] = dmat.astype(BF).reshape(32, 128, 512)
        in_maps.append(im)
    return in_maps


def kernel(**inputs) -> np.ndarray:
    from concourse.bass_utils import run_bass_kernel_spmd
    nc = build(1)
    in_maps = pack_inputs(inputs)
    res = run_bass_kernel_spmd(nc, in_maps, list(range(NCORES)))
    out = np.concatenate([np.asarray(res.results[i]["out"]) for i in range(NCORES)], axis=0)
    return out.astype(np.float32)


# revision 27
# speedup vs baseline: 29.6418x; 1.0073x over previous
"""Trainium2 Bass kernel for nn_AdvancedFeatureExtractor (8-core SPMD).

Decomposition (validated against the jax reference in numpy):
  - time conv branch: one shared 128-tap im2col matmul per 512-col chunk
    (all three kernel sizes packed into one (128,84) weight matrix);
    BN batch stats via bn_stats/bn_aggr + cross-core AllGather.
  - freq branch: direct DFT as matmul, frequency-sliced across cores
    (core m computes bins [256m, 256m+256) for ALL 64 samples = band m);
    mag/phase band means exchanged via the same AllGather.
  - wavelet branch: 4-level db4 DWT via block-diagonal strided-conv matmul.
  - fusion MLP: const channels (stats/freq/wavelet feats, constant over L)
    folded into a per-sample bias; only the 84 conv channels go through the
    big per-position matmuls.
"""
import sys
import os
import math

for _p in ("/opt/trn_rl_repo",):
    if _p not in sys.path:
        sys.path.insert(0, _p)

import numpy as np
import ml_dtypes

import concourse.bass as bass
import concourse.mybir as mybir
import concourse.tile as tile
from concourse.bass_types import AP
from concourse.vector_clock import ScopedClock

F32 = mybir.dt.float32
BF16 = mybir.dt.bfloat16
AF = mybir.ActivationFunctionType
OP = mybir.AluOpType
BF = ml_dtypes.bfloat16

L = 4094
B = 64
NCORES = 8
BS = B // NCORES            # 8 samples per core
CH = 512                    # free-dim chunk
NCH_S = 8                   # chunks per sample (last is 510 wide)
LASTW = L - (NCH_S - 1) * CH  # 510
NLOC = BS * L               # per-core BN element count = 32752
NGLOB = B * L               # 262016
BANDW = 256                 # bins per band (= per core)
PI = math.pi

# DWT level geometry
DWT_NIN = [4094, 2051, 1029, 518]
DWT_NOUT = [2051, 1029, 518, 263]
DWT_NPAD = [n + 14 for n in DWT_NIN]
DWT_WE = [(n + 1) // 2 for n in DWT_NPAD]   # even-index width
DWT_WO = [n // 2 for n in DWT_NPAD]         # odd-index width

_DB4_LO = np.array([-0.010597401784997278, 0.032883011666982945, 0.030841381835986965,
                    -0.18703481171888114, -0.02798376941698385, 0.6308807679295904,
                    0.7148465705525415, 0.23037781330885523], np.float32)
_DB4_HI = np.array([-0.23037781330885523, 0.7148465705525415, -0.6308807679295904,
                    -0.02798376941698385, 0.18703481171888114, 0.030841381835986965,
                    0.032883011666982945, -0.010597401784997278], np.float32)
WIN = (32, 64, 128)

_MAXW = 1


def _patched_drain_and_barrier(self, tick_clock, wait_clock):
    """walrus in this container rejects >1 sem wait on a CTRL instruction;
    split the Tile tail-drain waits across single-wait NOPs."""
    nc = self.nc
    probe = nc.sync.nop()
    wait_clock.add_sem_waits(probe.ins, ScopedClock({None: tick_clock.global_clock}))
    si = probe.ins.sync_info
    waits = list(si.on_wait) if si is not None else []
    updates = list(si.on_update) if si is not None else []
    probe.ins.sync_info = mybir.SyncInfo(on_wait=waits[:_MAXW], on_update=updates)
    for i in range(_MAXW, len(waits), _MAXW):
        nop = nc.sync.nop()
        nop.ins.sync_info = mybir.SyncInfo(on_wait=waits[i:i + _MAXW], on_update=[])
    nc.sync.drain()
    nc.all_engine_barrier()
    assert self.sems is not None
    popped = nc._tile_sem_poison_stack.pop()
    assert popped is self._sem_poison
    _sems = list(self.sems.allocated().values())
    _nums = sorted(s.num if hasattr(s, "num") else s for s in _sems)
    for _i in range(0, len(_nums), 8):
        chunk = _nums[_i:_i + 8]
        # only contiguous runs within the chunk
        run = [chunk[0]]
        for n in chunk[1:]:
            if n == run[-1] + 1:
                run.append(n)
            else:
                nc.gpsimd.dma_reset(range(run[0], run[-1] + 1))
                nc.gpsimd.sem_clear(range(run[0], run[-1] + 1))
                run = [n]
        nc.gpsimd.dma_reset(range(run[0], run[-1] + 1))
        nc.gpsimd.sem_clear(range(run[0], run[-1] + 1))
    nc._state.prepend_free_semaphores(_nums)
    for poison_set in nc._tile_sem_poison_stack:
        poison_set.update(_nums)
    nc.all_engine_barrier()


tile.TileContext._drain_and_barrier = _patched_drain_and_barrier


def _split_multi_waits(nc):
    """This walrus build allows only one sem-wait per instruction: hoist extra
    waits onto same-engine NOPs inserted immediately before the instruction."""
    for f in nc.m.functions:
        for bb in f.blocks:
            insts = bb.instructions
            i = 0
            while i < len(insts):
                ins = insts[i]
                si = getattr(ins, "sync_info", None)
                if si is not None and len(si.on_wait) > 1:
                    waits = list(si.on_wait)
                    for j, wsub in enumerate(waits[:-1]):
                        nop = mybir.InstNoOp(
                            name=f"{ins.name}.w{j}", engine=ins.engine,
                            bass_nofuse=True,
                            sync_info=mybir.SyncInfo(on_wait=[wsub], on_update=[]))
                        insts.insert(i, nop)
                        i += 1
                    ins.sync_info = mybir.SyncInfo(
                        on_wait=[waits[-1]], on_update=list(si.on_update))
                i += 1


def _ap(t, offset, dims):
    """Custom access pattern on a tile/param AP `t` (adds t's own offset)."""
    return AP(tensor=t.tensor, offset=t.offset + offset, ap=[list(d) for d in dims])


def build(n_iters: int = 1, phase: float = 5, split_waits: bool = True, bench: bool = False) -> bass.Bass:
    nc = bass.Bass()

    # ---- per-core external inputs ----
    xpad = nc.declare_dram_parameter("xpad", [BS, 4224], BF16, isOutput=False)
    xdwt = nc.declare_dram_parameter("xdwt", [BS, 4108], F32, isOutput=False)
    xt = nc.declare_dram_parameter("xt", [32, 128, 64], BF16, isOutput=False)
    dmat = nc.declare_dram_parameter("dmat", [32, 128, 512], BF16, isOutput=False)
    wall = nc.declare_dram_parameter("wall", [128, 84], BF16, isOutput=False)
    wallf = nc.declare_dram_parameter("wallf", [128, 84], F32, isOutput=False)
    wdwt = nc.declare_dram_parameter("wdwt", [64, 16], BF16, isOutput=False)
    w1v = nc.declare_dram_parameter("w1v", [84, 256], BF16, isOutput=False)
    w2p = nc.declare_dram_parameter("w2p", [128, 512], BF16, isOutput=False)
    w1cS = nc.declare_dram_parameter("w1cS", [28, 256], F32, isOutput=False)
    w1cM = nc.declare_dram_parameter("w1cM", [85, 256], F32, isOutput=False)
    w1cP = nc.declare_dram_parameter("w1cP", [85, 256], F32, isOutput=False)
    w1cW = nc.declare_dram_parameter("w1cW", [85, 256], F32, isOutput=False)
    fub1 = nc.declare_dram_parameter("fub1", [128, 2], F32, isOutput=False)
    fub2 = nc.declare_dram_parameter("fub2", [128, 2], F32, isOutput=False)
    stw = nc.declare_dram_parameter("stw", [4, 28], F32, isOutput=False)
    stb = nc.declare_dram_parameter("stb", [28], F32, isOutput=False)
    mlpw = {}
    for pre in ("mag", "ph", "wv"):
        mlpw[pre] = (
            nc.declare_dram_parameter(f"{pre}w1", [8, 85], F32, isOutput=False),
            nc.declare_dram_parameter(f"{pre}b1", [85], F32, isOutput=False),
            nc.declare_dram_parameter(f"{pre}w2", [85, 85], F32, isOutput=False),
            nc.declare_dram_parameter(f"{pre}b2", [85], F32, isOutput=False),
        )
    sel = nc.declare_dram_parameter("sel", [64, 8], F32, isOutput=False)
    eye = nc.declare_dram_parameter("eye", [128, 128], F32, isOutput=False)
    bng = nc.declare_dram_parameter("bng", [84], F32, isOutput=False)
    bnb = nc.declare_dram_parameter("bnb", [84], F32, isOutput=False)
    if bench:
        dummy = nc.declare_dram_parameter("bench_out", [1, 16], F32, isOutput=True)
    else:
        out_ext = nc.declare_dram_parameter("out", [BS, 256, L], F32, isOutput=True)
        dummy = None

    with tile.TileContext(nc) as tc:
        with tc.tile_pool(name="consts", bufs=1) as cpool, \
             tc.tile_pool(name="xstore", bufs=1) as xspool, \
             tc.tile_pool(name="dck", bufs=3) as dpool, \
             tc.tile_pool(name="ynp", bufs=4) as ynpool, \
             tc.tile_pool(name="h1p", bufs=6) as h1pool, \
             tc.tile_pool(name="stg", bufs=2) as stgpool, \
             tc.tile_pool(name="small", bufs=1) as spool, \
             tc.tile_pool(name="dwtsb", bufs=1) as wpool, \
             tc.tile_pool(name="ps", bufs=5, space="PSUM") as ps, \
             tc.tile_pool(name="psdft", bufs=1, space="PSUM") as psdft, \
             tc.tile_pool(name="psmlp", bufs=1, space="PSUM") as psmlp, \
             tc.tile_pool(name="dram", bufs=2, space="DRAM") as dram:

            # ================= constants =================
            wall_sb = cpool.tile([128, 84], BF16)
            nc.gpsimd.dma_start(wall_sb[:], wall[:])
            wallf_sb = cpool.tile([128, 84], F32)
            nc.gpsimd.dma_start(wallf_sb[:], wallf[:])
            xt_sb = cpool.tile([128, 2048], BF16)
            for c in range(32):
                nc.sync.dma_start(xt_sb[:, 64 * c:64 * c + 64], xt[c])
            w1v_sb = cpool.tile([84, 256], BF16)
            nc.gpsimd.dma_start(w1v_sb[:], w1v[:])
            w2_sb = cpool.tile([128, 512], BF16)
            nc.gpsimd.dma_start(w2_sb[:], w2p[:])
            w1cS_sb = cpool.tile([28, 256], F32)
            nc.gpsimd.dma_start(w1cS_sb[:], w1cS[:])
            w1cM_sb = cpool.tile([85, 256], F32)
            nc.gpsimd.dma_start(w1cM_sb[:], w1cM[:])
            w1cP_sb = cpool.tile([85, 256], F32)
            nc.gpsimd.dma_start(w1cP_sb[:], w1cP[:])
            w1cW_sb = cpool.tile([85, 256], F32)
            nc.gpsimd.dma_start(w1cW_sb[:], w1cW[:])
            fub1_sb = cpool.tile([128, 2], F32)
            nc.gpsimd.dma_start(fub1_sb[:], fub1[:])
            fub2_sb = cpool.tile([128, 2], F32)
            nc.gpsimd.dma_start(fub2_sb[:], fub2[:])
            stw_sb = cpool.tile([4, 28], F32)
            nc.gpsimd.dma_start(stw_sb[:], stw[:])
            stb_sb = cpool.tile([28, 1], F32)
            nc.gpsimd.dma_start(stb_sb[:], _ap(stb[:], 0, [[1, 28], [1, 1]]))
            mlp_sb = {}
            for pre in ("mag", "ph", "wv"):
                p1, pb1, p2, pb2 = mlpw[pre]
                w1_sb = cpool.tile([8, 85], F32, name=f"{pre}w1sb")
                nc.gpsimd.dma_start(w1_sb[:], p1[:])
                b1_sb = cpool.tile([85, 1], F32, name=f"{pre}b1sb")
                nc.gpsimd.dma_start(b1_sb[:], _ap(pb1[:], 0, [[1, 85], [1, 1]]))
                w2_sb2 = cpool.tile([85, 85], F32, name=f"{pre}w2sb")
                nc.gpsimd.dma_start(w2_sb2[:], p2[:])
                b2_sb = cpool.tile([85, 1], F32, name=f"{pre}b2sb")
                nc.gpsimd.dma_start(b2_sb[:], _ap(pb2[:], 0, [[1, 85], [1, 1]]))
                mlp_sb[pre] = (w1_sb, b1_sb, w2_sb2, b2_sb)
            bng_sb = cpool.tile([84, 1], F32)
            nc.gpsimd.dma_start(bng_sb[:], _ap(bng[:], 0, [[1, 84], [1, 1]]))
            bnb_sb = cpool.tile([84, 1], F32)
            nc.gpsimd.dma_start(bnb_sb[:], _ap(bnb[:], 0, [[1, 84], [1, 1]]))
            wdwt_sb = cpool.tile([64, 16], BF16)
            nc.gpsimd.dma_start(wdwt_sb[:], wdwt[:])
            ones8 = cpool.tile([8, 1], F32)
            nc.vector.memset(ones8[:], 1.0)
            sel_sb = cpool.tile([64, 8], F32)
            nc.gpsimd.dma_start(sel_sb[:], sel[:])
            eye_sb = cpool.tile([128, 128], F32)
            nc.gpsimd.dma_start(eye_sb[:], eye[:])

            if bench:
                out_ext = dram.tile([BS, 256, L], F32, name="out_bench", bufs=1)

            for it in range(n_iters):
                # ============ conv branch: im2col matmul + bn stats ============
                # X[p, s*4224 + j] = x[s, j + p - 64]; resident for reuse in fusion
                x_all = xspool.tile([128, BS * 4096], BF16)
                for s in range(BS):
                    nc.sync.dma_start(
                        x_all[:, s * 4096:(s + 1) * 4096],
                        _ap(xpad[:], s * 4224, [[1, 128], [1, 4096]]))
                bnstat = spool.tile([84, 6 * NCH_S * BS], F32)
                for s in range(BS):
                    for ci in range(NCH_S):
                        idx = s * NCH_S + ci
                        w = LASTW if ci == NCH_S - 1 else CH
                        yp = ps.tile([84, CH], F32, tag="bigps")
                        nc.tensor.matmul(yp[:], wall_sb[:],
                                         x_all[:, s * 4096 + ci * CH:s * 4096 + (ci + 1) * CH])
                        nc.vector.bn_stats(bnstat[:, idx * 6:idx * 6 + 6], yp[:, :w])
                bnmv = spool.tile([84, 2], F32)
                nc.vector.bn_aggr(bnmv[:], bnstat[:])

                if phase < 1.2:
                    continue
                # ============ DFT branch (own 256 bins, all 64 samples) ============
                dacc = psdft.tile([64, 512], F32)
                for c in range(32):
                    dt_ = dpool.tile([128, 512], BF16)
                    nc.sync.dma_start(dt_[:], dmat[c])
                    nc.tensor.matmul(dacc[:], xt_sb[:, c * 64:(c + 1) * 64], dt_[:],
                                     start=(c == 0), stop=(c == 31))
                if phase < 1.3:
                    continue
                bufs8 = [spool.tile([64, 256], F32, name=f"dp{i}") for i in range(8)]
                b1, b2, b3, b4, b5, b6, b7, b8 = [t[:] for t in bufs8]
                nc.vector.tensor_copy(b1, dacc[:, 0:256])   # re -> SBUF
                nc.vector.tensor_copy(b2, dacc[:, 256:512])  # im -> SBUF
                nc.vector.tensor_tensor(b3, b1, b1, OP.mult)
                nc.vector.tensor_tensor(b4, b2, b2, OP.mult)
                nc.vector.tensor_tensor(b5, b3, b4, OP.add)
                nc.scalar.sqrt(b6, b5)
                magm = spool.tile([64, 1], F32)
                nc.vector.tensor_reduce(magm[:], b6, mybir.AxisListType.X, OP.add)
                magmean = spool.tile([64, 1], F32)
                nc.scalar.mul(magmean[:], magm[:], 1.0 / BANDW)
                if phase < 1.6:
                    continue
                # atan2(im, re); arctan LUT domain is [-pi/2, pi/2] so range-reduce
                nc.scalar.activation(b3, b2, AF.Abs)  # |im|
                nc.scalar.activation(b4, b1, AF.Abs)  # |re|
                nc.vector.tensor_tensor(b5, b3, b4, OP.min)
                nc.vector.tensor_tensor(b6, b3, b4, OP.max)
                nc.vector.reciprocal(b7, b6)
                nc.vector.tensor_tensor(b8, b5, b7, OP.mult)
                nc.scalar.activation(b5, b8, AF.Arctan)
                nc.vector.tensor_tensor(b6, b3, b4, OP.is_gt)   # swap flag
                nc.vector.tensor_scalar(b7, b6, -2.0, 1.0, OP.mult, OP.add)
                nc.vector.tensor_tensor(b8, b5, b7, OP.mult)
                nc.vector.scalar_tensor_tensor(b7, b6, PI / 2, b8, OP.mult, OP.add)  # ang0
                nc.vector.tensor_scalar(b3, b1, 0.0, None, OP.is_lt)   # re<0
                nc.vector.tensor_scalar(b4, b3, -2.0, 1.0, OP.mult, OP.add)
                nc.vector.tensor_tensor(b5, b7, b4, OP.mult)
                nc.vector.scalar_tensor_tensor(b6, b3, PI, b5, OP.mult, OP.add)      # ang1
                nc.vector.tensor_scalar(b4, b2, 0.0, None, OP.is_ge)
                nc.vector.tensor_scalar(b5, b4, 2.0, -1.0, OP.mult, OP.add)          # sign(im)
                nc.vector.tensor_tensor(b8, b6, b5, OP.mult)
                phm = spool.tile([64, 1], F32)
                nc.vector.tensor_reduce(phm[:], b8, mybir.AxisListType.X, OP.add)
                phmean = spool.tile([64, 1], F32)
                nc.scalar.mul(phmean[:], phm[:], 1.0 / BANDW)

                # ============ BN partial sums -> AllGather row ============
                bsum = spool.tile([84, 1], F32)
                nc.scalar.mul(bsum[:], bnmv[:, 0:1], float(NLOC))
                msq_ = spool.tile([84, 1], F32)
                nc.vector.tensor_tensor(msq_[:], bnmv[:, 0:1], bnmv[:, 0:1], OP.mult)
                vps = spool.tile([84, 1], F32)
                nc.vector.tensor_tensor(vps[:], bnmv[:, 1:2], msq_[:], OP.add)
                bsq = spool.tile([84, 1], F32)
                nc.vector.tensor_scalar(bsq[:], vps[:], float(NLOC), None, OP.mult)

                ag_src = dram.tile([1, 296], F32)
                ag_dst = dram.tile([8, 296], F32, addr_space="Shared")
                nc.gpsimd.dma_start(_ap(ag_src[:], 0, [[1, 64], [1, 1]]), magmean[:])
                nc.gpsimd.dma_start(_ap(ag_src[:], 64, [[1, 64], [1, 1]]), phmean[:])
                nc.gpsimd.dma_start(_ap(ag_src[:], 128, [[1, 84], [1, 1]]), bsum[:])
                nc.gpsimd.dma_start(_ap(ag_src[:], 212, [[1, 84], [1, 1]]), bsq[:])

                # ============ x-stats + DWT (overlap with collective) ============
                apad0 = wpool.tile([BS, 4108], F32, name="apad0")
                nc.gpsimd.dma_start(apad0[:], xdwt[:])
                xrow = apad0[:, 7:7 + L]
                xjunk = wpool.tile([BS, L], BF16, name="xjunk")
                xsum = spool.tile([BS, 1], F32)
                nc.scalar.activation(xjunk[:], xrow, AF.Copy, accum_out=xsum[:])
                xsqs = spool.tile([BS, 1], F32)
                nc.scalar.activation(xjunk[:], xrow, AF.Square, accum_out=xsqs[:])
                xmax = spool.tile([BS, 1], F32)
                nc.vector.tensor_reduce(xmax[:], xrow, mybir.AxisListType.X, OP.max)
                xmin = spool.tile([BS, 1], F32)
                nc.vector.tensor_reduce(xmin[:], xrow, mybir.AxisListType.X, OP.min)
                xmean = spool.tile([BS, 1], F32)
                nc.scalar.mul(xmean[:], xsum[:], 1.0 / L)
                xmsq = spool.tile([BS, 1], F32)
                nc.vector.tensor_tensor(xmsq[:], xmean[:], xmean[:], OP.mult)
                xu = spool.tile([BS, 1], F32)
                nc.vector.tensor_scalar(xu[:], xmsq[:], -float(L) / (L - 1), None, OP.mult)
                xv_ = spool.tile([BS, 1], F32)
                nc.vector.scalar_tensor_tensor(xv_[:], xsqs[:], 1.0 / (L - 1), xu[:], OP.mult, OP.add)
                xstd = spool.tile([BS, 1], F32)
                nc.scalar.sqrt(xstd[:], xv_[:])
                statdram = dram.tile([4, 8], F32)
                for r, tl in enumerate((xmean, xstd, xmax, xmin)):
                    nc.gpsimd.dma_start(_ap(statdram[:], 8 * r, [[1, 8], [1, 1]]), tl[:])

                # DWT levels
                wf_dram = dram.tile([8, 16], F32)
                apad_cur = apad0  # (8, 4108) level-0 padded input
                for lv in range(4):
                    nin, nout = DWT_NIN[lv], DWT_NOUT[lv]
                    npad, we, wo = DWT_NPAD[lv], DWT_WE[lv], DWT_WO[lv]
                    nrow = BS if lv == 0 else 16
                    # deinterleave into even/odd, bounce through DRAM
                    ae = wpool.tile([nrow, we], BF16, name=f"ae{lv}", tag="aet")
                    nc.scalar.copy(
                        ae[:], _ap(apad_cur[:], 0, [[apad_cur.ap[0][0], nrow], [2, we]]))
                    ao = wpool.tile([nrow, wo], BF16, name=f"ao{lv}", tag="aot")
                    nc.scalar.copy(
                        ao[:], _ap(apad_cur[:], 1, [[apad_cur.ap[0][0], nrow], [2, wo]]))
                    ae_d = dram.tile([nrow, we], BF16, name=f"aed{lv}")
                    nc.gpsimd.dma_start(ae_d[:], ae[:])
                    ao_d = dram.tile([nrow, wo], BF16, name=f"aod{lv}")
                    nc.gpsimd.dma_start(ao_d[:], ao[:])
                    xlv = wpool.tile([64, nout], BF16, name=f"xlv{lv}", tag="xlv")
                    rstep = we if lv == 0 else 2 * we
                    rstepo = wo if lv == 0 else 2 * wo
                    for t in range(8):
                        srcd = ae_d if t % 2 == 0 else ao_d
                        rst = rstep if t % 2 == 0 else rstepo
                        eng = nc.sync if t % 4 < 2 else nc.gpsimd
                        eng.dma_start(
                            xlv[8 * t:8 * t + 8, :],
                            _ap(srcd[:], t // 2, [[rst, 8], [1, nout]]))
                    # next-level padded buffer (evac target)
                    nch = (nout + CH - 1) // CH
                    if lv < 3:
                        apad_next = wpool.tile([16, DWT_NPAD[lv + 1]], F32, name=f"apad{lv + 1}", tag="apadA" if (lv + 1) % 2 == 0 else "apadB")
                        evtgt = apad_next
                        evoff = 7
                    else:
                        evtgt = wpool.tile([16, nout], F32, name="apadj", tag="apadA")
                        evoff = 0
                    csums = spool.tile([16, 8], F32, name=f"cs{lv}")
                    cabss = spool.tile([16, 8], F32, name=f"ca{lv}")
                    csqs = spool.tile([16, 8], F32, name=f"cq{lv}")
                    for c in range(nch):
                        c0 = c * CH
                        wch = min(CH, nout - c0)
                        dp = ps.tile([16, CH], F32, tag="bigps", name=f"dwtp{lv}")
                        nc.tensor.matmul(dp[:, :wch], wdwt_sb[:], xlv[:, c0:c0 + wch])
                        nc.scalar.activation(evtgt[:, evoff + c0:evoff + c0 + wch],
                                             dp[:, :wch], AF.Copy,
                                             accum_out=csums[:, c:c + 1])
                        junka = wpool.tile([16, CH], BF16, name="junka")
                        nc.scalar.activation(junka[:, :wch], dp[:, :wch], AF.Abs,
                                             accum_out=cabss[:, c:c + 1])
                        junkb = wpool.tile([16, CH], F32, name="junkb")
                        cad = evtgt[:, evoff + c0:evoff + c0 + wch]
                        nc.vector.tensor_tensor(junkb[:, :wch], cad, cad, OP.mult)
                        nc.vector.tensor_reduce(csqs[:, c:c + 1], junkb[:, :wch],
                                                mybir.AxisListType.X, OP.add)
                    if lv < 3:
                        # symmetric pads for next level
                        npn = DWT_NPAD[lv + 1]
                        nc.vector.tensor_copy(
                            apad_next[:, 0:7],
                            _ap(apad_next[:], 13, [[apad_next.ap[0][0], 16], [-1, 7]]))
                        nc.vector.tensor_copy(
                            apad_next[:, 7 + nout:npn],
                            _ap(apad_next[:], 7 + nout - 1,
                                [[apad_next.ap[0][0], 16], [-1, 7]]))
                        apad_cur = apad_next
                    # reduce chunk stats -> level features
                    ctot = spool.tile([16, 1], F32, name=f"ct{lv}")
                    nc.vector.tensor_reduce(ctot[:], csums[:, :nch], mybir.AxisListType.X, OP.add)
                    atot = spool.tile([16, 1], F32, name=f"at{lv}")
                    nc.vector.tensor_reduce(atot[:], cabss[:, :nch], mybir.AxisListType.X, OP.add)
                    qtot = spool.tile([16, 1], F32, name=f"qt{lv}")
                    nc.vector.tensor_reduce(qtot[:], csqs[:, :nch], mybir.AxisListType.X, OP.add)
                    man = spool.tile([16, 1], F32, name=f"man{lv}")
                    nc.scalar.mul(man[:], atot[:], 1.0 / nout)
                    cmean = spool.tile([16, 1], F32, name=f"cm{lv}")
                    nc.scalar.mul(cmean[:], ctot[:], 1.0 / nout)
                    cmsq = spool.tile([16, 1], F32, name=f"cmq{lv}")
                    nc.vector.tensor_tensor(cmsq[:], cmean[:], cmean[:], OP.mult)
                    cvar = spool.tile([16, 1], F32, name=f"cv{lv}")
                    nc.vector.scalar_tensor_tensor(cvar[:], qtot[:], 1.0 / nout, cmsq[:],
                                                   OP.mult, OP.subtract)
                    cstd = spool.tile([16, 1], F32, name=f"cd{lv}")
                    nc.scalar.sqrt(cstd[:], cvar[:])
                    r0 = 2 * (3 - lv)
                    nc.gpsimd.dma_start(_ap(wf_dram[:], 16 * r0, [[1, 16], [1, 1]]), man[:])
                    nc.gpsimd.dma_start(_ap(wf_dram[:], 16 * (r0 + 1), [[1, 16], [1, 1]]), cstd[:])

                # ============ collective ============
                nc.gpsimd.collective_compute(
                    "AllGather", OP.bypass,
                    replica_groups=[list(range(NCORES))],
                    ins=[ag_src.opt()], outs=[ag_dst.opt()])

                # ============ post-collective: BN + const features ============
                gath = spool.tile([8, 296], F32)
                nc.gpsimd.dma_start(gath[:], ag_dst[:])
                gsp = psmlp.tile([1, 296], F32, tag="mlpp")
                nc.tensor.matmul(gsp[:], ones8[:], gath[:]) 
                gsums = spool.tile([1, 296], F32)
                nc.vector.tensor_copy(gsums[:], gsp[:])
                gb_d = dram.tile([1, 296], F32)
                nc.gpsimd.dma_start(gb_d[:], gsums[:])
                gs84 = spool.tile([84, 1], F32)
                nc.gpsimd.dma_start(gs84[:], _ap(gb_d[:], 128, [[1, 84], [1, 1]]))
                gq84 = spool.tile([84, 1], F32)
                nc.gpsimd.dma_start(gq84[:], _ap(gb_d[:], 212, [[1, 84], [1, 1]]))
                bmean = spool.tile([84, 1], F32)
                nc.scalar.mul(bmean[:], gs84[:], 1.0 / NGLOB)
                bmneg = spool.tile([84, 1], F32)
                nc.scalar.mul(bmneg[:], gs84[:], -1.0 / NGLOB)
                bmsq = spool.tile([84, 1], F32)
                nc.vector.tensor_tensor(bmsq[:], bmean[:], bmneg[:], OP.mult)
                bvar = spool.tile([84, 1], F32)
                nc.vector.scalar_tensor_tensor(bvar[:], gq84[:], 1.0 / NGLOB, bmsq[:],
                                               OP.mult, OP.add)
                bve = spool.tile([84, 1], F32)
                nc.vector.tensor_scalar(bve[:], bvar[:], 1e-5, None, OP.add)
                bsd = spool.tile([84, 1], F32)
                nc.scalar.sqrt(bsd[:], bve[:])
                brq = spool.tile([84, 1], F32)
                nc.vector.reciprocal(brq[:], bsd[:])
                bnscale = spool.tile([84, 1], F32)
                nc.vector.tensor_tensor(bnscale[:], brq[:], bng_sb[:], OP.mult)
                bnms = spool.tile([84, 1], F32)
                nc.vector.tensor_tensor(bnms[:], bmneg[:], bnscale[:], OP.mult)
                bnbias = spool.tile([84, 1], F32)
                nc.vector.tensor_tensor(bnbias[:], bnms[:], bnb_sb[:], OP.add)
                # fold BN scale into conv weights: W_bn = W * diag(bnscale)
                zdiag = spool.tile([84, 84], F32, name="zdiag")
                nc.vector.memset(zdiag[:], 0.0)
                diag_d = dram.tile([84, 84], F32, name="diagd")
                nc.gpsimd.dma_start(diag_d[:], zdiag[:])
                nc.gpsimd.dma_start(_ap(diag_d[:], 0, [[85, 84], [1, 1]]), bnscale[:])
                diag_sb = spool.tile([84, 84], F32, name="diagsb")
                nc.gpsimd.dma_start(diag_sb[:], diag_d[:])
                wtp = psmlp.tile([84, 128], F32, tag="mlpp", name="wtp")
                nc.tensor.transpose(wtp[:], wallf_sb[:], eye_sb[:])
                wt_sb = spool.tile([84, 128], F32, name="wtsb")
                nc.vector.tensor_copy(wt_sb[:], wtp[:])
                wbnp = psmlp.tile([128, 84], F32, tag="mlpp", name="wbnp")
                nc.tensor.matmul(wbnp[:], wt_sb[:], diag_sb[:])
                wbn_sb = spool.tile([128, 84], BF16, name="wbnsb")
                nc.scalar.copy(wbn_sb[:], wbnp[:])

                statsT = spool.tile([4, 8], F32)
                nc.gpsimd.dma_start(statsT[:], statdram[:])
                wfT = spool.tile([8, 8], F32)
                for s_ in range(8):
                    nc.gpsimd.dma_start(
                        wfT[:, s_:s_ + 1],
                        _ap(wf_dram[:], 2 * s_ + 1, [[16, 8], [1, 1]]))

                cfS = spool.tile([28, 8], F32)
                sfp = psmlp.tile([28, 8], F32, tag="mlpp")
                nc.tensor.matmul(sfp[:], stw_sb[:], statsT[:])
                nc.scalar.activation(cfS[:], sfp[:], AF.Identity, bias=stb_sb[:])

                # band MLPs: all-64-sample layer-1, select own samples via
                # one-hot matmul (selection commutes with bias/relu), then
                # PE-transpose so biases become per-partition.
                cfM = spool.tile([85, 8], F32)
                cfP = spool.tile([85, 8], F32)
                cfW = spool.tile([85, 8], F32)
                for pre, goff, tgt in (("mag", 0, cfM), ("ph", 64, cfP)):
                    w1_sb, b1_sb, w2_sb2, b2_sb = mlp_sb[pre]
                    h64p = psmlp.tile([64, 85], F32, tag="mlpp", name=f"h64p{pre}")
                    nc.tensor.matmul(h64p[:], gath[0:8, goff:goff + 64], w1_sb[:])
                    h64 = spool.tile([64, 85], F32, name=f"h64{pre}")
                    nc.vector.tensor_copy(h64[:], h64p[:])
                    hselp = psmlp.tile([8, 85], F32, tag="mlpp", name=f"hsel{pre}")
                    nc.tensor.matmul(hselp[:], sel_sb[:], h64[:])
                    hsel = spool.tile([8, 85], F32, name=f"hselS{pre}")
                    nc.vector.tensor_copy(hsel[:], hselp[:])
                    htp = psmlp.tile([85, 8], F32, tag="mlpp", name=f"htp{pre}")
                    nc.tensor.transpose(htp[:], hsel[:], eye_sb[0:8, 0:8])
                    ht = spool.tile([85, 8], F32, name=f"ht{pre}")
                    nc.scalar.activation(ht[:], htp[:], AF.Relu, bias=b1_sb[:])
                    op2 = psmlp.tile([85, 8], F32, tag="mlpp", name=f"op{pre}")
                    nc.tensor.matmul(op2[:], w2_sb2[:], ht[:])
                    nc.scalar.activation(tgt[:], op2[:], AF.Identity, bias=b2_sb[:])
                # wavelet MLP (local samples, simple orientation)
                w1_sb, b1_sb, w2_sb2, b2_sb = mlp_sb["wv"]
                hpw = psmlp.tile([85, 8], F32, tag="mlpp", name="hpwv")
                nc.tensor.matmul(hpw[:], w1_sb[:], wfT[:])
                hhw = spool.tile([85, 8], F32, name="hhwv")
                nc.scalar.activation(hhw[:], hpw[:], AF.Relu, bias=b1_sb[:])
                opw = psmlp.tile([85, 8], F32, tag="mlpp", name="opwv")
                nc.tensor.matmul(opw[:], w2_sb2[:], hhw[:])
                nc.scalar.activation(cfW[:], opw[:], AF.Identity, bias=b2_sb[:])
                # const-channel contribution to layer-1 bias
                cbT = []
                for oh in range(2):
                    cbp = psmlp.tile([128, 8], F32, tag="mlpp", name=f"cbp{oh}")
                    sl = slice(oh * 128, oh * 128 + 128)
                    nc.tensor.matmul(cbp[:], w1cS_sb[:, sl], cfS[:], start=True, stop=False)
                    nc.tensor.matmul(cbp[:], w1cM_sb[:, sl], cfM[:], start=False, stop=False)
                    nc.tensor.matmul(cbp[:], w1cP_sb[:, sl], cfP[:], start=False, stop=False)
                    nc.tensor.matmul(cbp[:], w1cW_sb[:, sl], cfW[:], start=False, stop=True)
                    cb = spool.tile([128, 8], F32, name=f"cbT{oh}")
                    nc.scalar.activation(cb[:], cbp[:], AF.Identity,
                                         bias=fub1_sb[:, oh:oh + 1])
                    cbT.append(cb)

                if phase < 5:
                    continue
                # ============ fusion ============
                HB = 2048  # staged output columns per DMA
                for s in range(BS):
                    for half in range(2):
                        st0 = stgpool.tile([128, HB], F32, name="st0")
                        st1 = stgpool.tile([128, HB], F32, name="st1")
                        for ci4 in range(4):
                            ci = half * 4 + ci4
                            zp = psmlp.tile([84, CH], F32, tag="zpp", name="zp")
                            nc.tensor.matmul(
                                zp[:], wbn_sb[:],
                                x_all[:, s * 4096 + ci * CH:s * 4096 + (ci + 1) * CH])
                            yn = ynpool.tile([84, CH], BF16)
                            nc.scalar.activation(yn[:], zp[:], AF.Relu, bias=bnbias[:])
                            h1s = []
                            for oh in range(2):
                                hp1 = ps.tile([128, CH], F32, tag="bigps", name=f"hps{oh}")
                                nc.tensor.matmul(hp1[:], w1v_sb[:, oh * 128:(oh + 1) * 128], yn[:])
                                h1 = h1pool.tile([128, CH], BF16, name=f"h1{oh}")
                                if oh == 0:
                                    nc.scalar.activation(h1[:], hp1[:], AF.Relu,
                                                         bias=cbT[0][:, s:s + 1])
                                else:
                                    nc.vector.tensor_scalar(h1[:], hp1[:], cbT[1][:, s:s + 1],
                                                            0.0, OP.add, OP.max)
                                h1s.append(h1)
                            for of in range(2):
                                op_ = ps.tile([128, CH], F32, tag="bigps", name=f"ops{of}")
                                nc.tensor.matmul(op_[:], w2_sb[:, (0 * 2 + of) * 128:(0 * 2 + of) * 128 + 128],
                                                 h1s[0][:], start=True, stop=False)
                                nc.tensor.matmul(op_[:], w2_sb[:, (1 * 2 + of) * 128:(1 * 2 + of) * 128 + 128],
                                                 h1s[1][:], start=False, stop=True)
                                tgt = (st0 if of == 0 else st1)[:, ci4 * CH:(ci4 + 1) * CH]
                                nc.vector.tensor_scalar(tgt, op_[:], fub2_sb[:, of:of + 1],
                                                        None, OP.add)
                        w2c = HB if half == 0 else (L - HB)
                        for of, st in ((0, st0), (1, st1)):
                            nc.sync.dma_start(
                                _ap(out_ext[:], s * 256 * L + (of * 128) * L + half * HB,
                                    [[L, 128], [1, w2c]]),
                                st[:, :w2c])
            if bench:
                dnull = spool.tile([1, 16], F32, name="dnull")
                nc.gpsimd.dma_start(dnull[:], _ap(out_ext[:], 0, [[16, 1], [1, 16]]))
                nc.gpsimd.dma_start(dummy[:], dnull[:])
    if split_waits:
        _split_multi_waits(nc)
    return nc


def pack_inputs(inputs: dict) -> list[dict]:
    x = np.asarray(inputs["x"], np.float32)
    fu_w1 = np.asarray(inputs["fu_w1"], np.float32)
    fu_w2 = np.asarray(inputs["fu_w2"], np.float32)

    # shared (replicated) tensors
    wall = np.zeros((128, 84), np.float32)
    for i, k in enumerate(WIN):
        w = np.asarray(inputs[f"tc_w{i}"], np.float32)[:, 0, :]  # (28, k)
        p0 = 64 - k // 2
        wall[p0:p0 + k, i * 28:(i + 1) * 28] = w.T
    wallf32 = wall.copy()
    wall = wall.astype(BF)

    lo = _DB4_LO[::-1].copy()
    hi = _DB4_HI[::-1].copy()
    wdwt = np.zeros((64, 16), np.float32)
    for t in range(8):
        for s in range(8):
            wdwt[8 * t + s, 2 * s] = lo[t]
            wdwt[8 * t + s, 2 * s + 1] = hi[t]

    xtfull = np.zeros((4096, 64), np.float32)
    xtfull[:L] = x.T
    xt = xtfull.astype(BF).reshape(32, 128, 64)

    w1v = fu_w1[:84].astype(BF)                        # (84, 256)
    w2p = np.zeros((128, 512), np.float32)
    for kh in range(2):
        for oh in range(2):
            w2p[:, (kh * 2 + oh) * 128:(kh * 2 + oh) * 128 + 128] = \
                fu_w2[kh * 128:(kh + 1) * 128, oh * 128:(oh + 1) * 128]
    w2p = w2p.astype(BF)

    w1cS = fu_w1[84:112].copy()     # sf
    w1cM = fu_w1[112:197].copy()    # mag
    w1cP = fu_w1[197:282].copy()    # ph
    w1cW = fu_w1[282:367].copy()    # wf
    fub1 = np.stack([np.asarray(inputs["fu_b1"], np.float32)[:128],
                     np.asarray(inputs["fu_b1"], np.float32)[128:]], axis=1)
    fub2 = np.stack([np.asarray(inputs["fu_b2"], np.float32)[:128],
                     np.asarray(inputs["fu_b2"], np.float32)[128:]], axis=1)

    shared = {
        "eye": np.eye(128, dtype=np.float32),
        "xt": xt, "wall": wall, "wallf": wallf32, "wdwt": wdwt.astype(BF),
        "w1v": w1v, "w2p": w2p, "w1cS": w1cS, "w1cM": w1cM, "w1cP": w1cP, "w1cW": w1cW,
        "fub1": fub1, "fub2": fub2,
        "stw": np.asarray(inputs["st_w"], np.float32),
        "stb": np.asarray(inputs["st_b"], np.float32),
        "bng": np.concatenate([np.asarray(inputs[f"bn_g{i}"], np.float32) for i in range(3)]),
        "bnb": np.concatenate([np.asarray(inputs[f"bn_b{i}"], np.float32) for i in range(3)]),
    }
    for pre in ("mag", "ph", "wv"):
        shared[f"{pre}w1"] = np.asarray(inputs[f"{pre}_w1"], np.float32)
        shared[f"{pre}b1"] = np.asarray(inputs[f"{pre}_b1"], np.float32)
        shared[f"{pre}w2"] = np.asarray(inputs[f"{pre}_w2"], np.float32)
        shared[f"{pre}b2"] = np.asarray(inputs[f"{pre}_b2"], np.float32)

    lidx = np.arange(4096, dtype=np.float64)
    in_maps = []
    for m in range(NCORES):
        xs = x[m * BS:(m + 1) * BS]
        xpad = np.zeros((BS, 4224), np.float32)
        xpad[:, 64:64 + L] = xs
        xdwt = np.zeros((BS, 4108), np.float32)
        xdwt[:, 7:7 + L] = xs
        xdwt[:, 0:7] = xs[:, 6::-1]
        xdwt[:, 7 + L:] = xs[:, L - 1:L - 8:-1]
        bins = np.arange(256 * m, 256 * m + 256, dtype=np.float64)
        ang = -2.0 * np.pi * np.outer(lidx, bins) / L
        dmat = np.zeros((4096, 512), np.float32)
        dmat[:L, 0:256] = np.cos(ang[:L])
        dmat[:L, 256:512] = np.sin(ang[:L])
        selm = np.zeros((64, 8), np.float32)
        for s in range(BS):
            selm[m * BS + s, s] = 1.0
        im = dict(shared)
        im["sel"] = selm
        im["xpad"] = xpad.astype(BF)
        im["xdwt"] = xdwt
        im["dmat"] = dmat.astype(BF).reshape(32, 128, 512)
        in_maps.append(im)
    return in_maps


def kernel(**inputs) -> np.ndarray:
    from concourse.bass_utils import run_bass_kernel_spmd
    nc = build(1)
    in_maps = pack_inputs(inputs)
    res = run_bass_kernel_spmd(nc, in_maps, list(range(NCORES)))
    out = np.concatenate([np.asarray(res.results[i]["out"]) for i in range(NCORES)], axis=0)
    return out.astype(np.float32)
